# revision 1
# baseline (speedup 1.0000x reference)
"""BitAttention TRN2 kernel: 8-core SPMD (DP over batch x TP over kv-heads).

Self-contained: hardcodes shapes B=2, S=2048, D=2048, H=16, KH=4.
Core r: batch b = r//4, kv-head kh = r%4, output token-quarter q# = r%4.

Math (forward-equivalent to the reference):
  - linear_bit = rms_norm -> per-row int8 act quant -> ternary weight quant -> matmul.
    Activations quantize to integers in [-127,127] (exact in bf16); ternary weights
    in {-1,0,1} (exact in bf16) -> projections run as exact-integer bf16 matmuls,
    dequant scales applied at PSUM eviction.
  - The reference einsum sums the query-head group axis, so Q's 16 heads collapse
    to 4 effective heads: group-sum the ternary w_q rows (ints in [-4,4], exact).
  - Both /sqrt(HD) scalings fold into one exact *(1/128) on q.
  - Attention (scores, softmax, P@V) runs in f32.
  - RoPE even/odd pairs are made contiguous by permuting w_q/w_k output dims
    (scores are invariant to a shared permutation of q/k feature dims).
"""
import numpy as np
from contextlib import ExitStack

import concourse.bass as bass
import concourse.bacc as bacc
import concourse.mybir as mybir
import concourse.tile as tile
from concourse.bass_utils import run_bass_kernel_spmd
from concourse.masks import make_identity, make_causal_mask

B, S, D = 2, 2048, 2048
H, KH = 16, 4
HD = D // H          # 128
KVD = KH * HD        # 512
NB = S // 128        # 16 token blocks
SQ = S // 4          # 512 tokens per output quarter
EPS = 1e-8
MAGIC = float(1.5 * 2 ** 23)
ATANH05 = 0.5493061443340549      # arctanh(0.5)
NEG = -3.4e38
F32 = mybir.dt.float32
BF16 = mybir.dt.bfloat16
AX = mybir.AxisListType
OP = mybir.AluOpType
AF = mybir.ActivationFunctionType

_cache = {}


def build(causal: bool, local_cc: bool = False):
    nc = bacc.Bacc()
    x_d = nc.dram_tensor("x", [S, D], F32, kind="ExternalInput")
    wq_d = nc.dram_tensor("wq", [D, KVD], F32, kind="ExternalInput")   # selected+perm+T
    wk_d = nc.dram_tensor("wk", [D, HD], F32, kind="ExternalInput")    # perm+T
    wv_d = nc.dram_tensor("wv", [D, HD], F32, kind="ExternalInput")    # T
    wo_d = nc.dram_tensor("wo", [KVD, D], F32, kind="ExternalInput")   # w_o.T full
    cos_d = nc.dram_tensor("cos", [S, HD // 2], F32, kind="ExternalInput")
    sin_d = nc.dram_tensor("sin", [S, HD // 2], F32, kind="ExternalInput")
    qsel_d = nc.dram_tensor("qsel", [128, 2], F32, kind="ExternalInput")  # quad one-hot
    y_d = nc.dram_tensor("y", [SQ, D], F32, kind="ExternalOutput")
    st_in = nc.dram_tensor("st_in", [1, 4], F32)
    st_out = nc.dram_tensor("st_out", [1, 4], F32, addr_space="Shared")
    cc_in = nc.dram_tensor("cc_in", [8, SQ, HD], F32)
    cc_out = nc.dram_tensor("cc_out", [8, SQ, HD], F32)

    with tile.TileContext(nc) as tc, ExitStack() as ctx:
        cpool = ctx.enter_context(tc.tile_pool(name="const", bufs=1))
        sm = ctx.enter_context(tc.tile_pool(name="sm", bufs=1))
        wint = ctx.enter_context(tc.tile_pool(name="wint", bufs=1))
        psmm = ctx.enter_context(tc.tile_pool(name="psmm", bufs=3, space="PSUM"))
        pstp = ctx.enter_context(tc.tile_pool(name="pstp", bufs=2, space="PSUM"))

        # ---------- constants ----------
        idf = cpool.tile([128, 128], F32, tag="idf")
        make_identity(nc, idf[:])
        idb = cpool.tile([128, 128], BF16, tag="idb")
        make_identity(nc, idb[:])
        eps_t = cpool.tile([128, 1], F32, tag="eps")
        nc.any.memset(eps_t[:], EPS)
        c127 = cpool.tile([128, 1], F32, tag="c127")
        nc.any.memset(c127[:], 127.0)
        ones_c = cpool.tile([128, 1], F32, tag="onc")
        nc.any.memset(ones_c[:], 1.0)
        ones_r = cpool.tile([1, 128], F32, tag="onr")
        nc.any.memset(ones_r[:], 1.0)
        inv_n = cpool.tile([128, 4], F32, tag="invn")
        for j, numel in enumerate([D * D, KVD * D, KVD * D, D * KVD]):
            nc.any.memset(inv_n[:, j:j + 1], 1.0 / (2.0 * numel))
        cmask = cpool.tile([128, 128], F32, tag="cmask")
        if causal:
            make_causal_mask(nc, cmask[:], mask_val=NEG)
        cos_all = cpool.tile([128, NB, HD // 2], F32, tag="cosall")
        sin_all = cpool.tile([128, NB, HD // 2], F32, tag="sinall")
        nc.sync.dma_start(cos_all[:], cos_d.ap().rearrange("(i p) f -> p i f", p=128))
        nc.sync.dma_start(sin_all[:], sin_d.ap().rearrange("(i p) f -> p i f", p=128))

        # persistent small tiles
        deq_all = sm.tile([128, NB], F32, tag="deq_all")
        partials = sm.tile([128, 52], F32, tag="partials")
        ptot = sm.tile([128, 4], F32, tag="ptot")
        st_sb = sm.tile([1, 4], F32, tag="st_sb")
        st2_sb = sm.tile([1, 4], F32, tag="st2_sb")
        totals = sm.tile([128, 4], F32, tag="totals")
        s4 = sm.tile([128, 4], F32, tag="s4")
        thr4 = sm.tile([128, 4], F32, tag="thr4")
        a4 = sm.tile([128, 4], F32, tag="a4")
        aq128 = sm.tile([128, 1], F32, tag="aq128")

        # int weights (persistent)
        wqkv_i = [wint.tile([128, 3 * HD], BF16, tag=f"wi{j}", name=f"wi{j}") for j in range(NB)]
        wo_i = [wint.tile([128, D], BF16, tag=f"wo{c}", name=f"wo{c}") for c in range(4)]

        # ---------- weights pass 1: |w| partial row-sums ----------
        with tc.tile_pool(name="wstream", bufs=4) as wstream:
            def rsum(dst_col, src_ap, w, tagsfx):
                t = wstream.tile([128, w], F32, tag="wst")
                nc.sync.dma_start(t[:, :w], src_ap)
                nc.vector.tensor_reduce(partials[:, dst_col:dst_col + 1], t[:, :w],
                                        axis=AX.X, op=OP.add, apply_absolute_value=True)

            for j in range(NB):
                rsum(j, wq_d[j * 128:(j + 1) * 128, :], KVD, f"q{j}")
            for j in range(NB):
                rsum(16 + j, wk_d[j * 128:(j + 1) * 128, :], HD, f"k{j}")
            for j in range(NB):
                rsum(32 + j, wv_d[j * 128:(j + 1) * 128, :], HD, f"v{j}")
            # w_o: this core's quarter of output dims = columns via qsel mask later;
            # simpler: sum our quarter rows of woT columns [kh*512:(kh+1)*512] is not
            # expressible core-dependently -> host zeroes other quarters? No: host
            # passes identical woT; quarter selection done via per-core input "qsel"
            # would complicate. Instead: every core sums ALL of woT and we divide by
            # 8 (each element counted once per core).
            for c in range(4):
                rsum(48 + c, wo_d[c * 128:(c + 1) * 128, :], D, f"o{c}")

            # segment reductions -> ptot [128,4]
            nc.vector.tensor_reduce(ptot[:, 0:1], partials[:, 0:16], axis=AX.X, op=OP.add)
            nc.vector.tensor_reduce(ptot[:, 1:2], partials[:, 16:32], axis=AX.X, op=OP.add)
            nc.vector.tensor_reduce(ptot[:, 2:3], partials[:, 32:48], axis=AX.X, op=OP.add)
            nc.vector.tensor_reduce(ptot[:, 3:4], partials[:, 48:52], axis=AX.X, op=OP.add)
            # w_o was summed fully on every core: scale its partial by 1/4 so the
            # 8-core AllReduce total equals 2x full-sum like the others
            nc.vector.tensor_scalar(ptot[:, 3:4], ptot[:, 3:4], 0.25, None, op0=OP.mult)
            pcol = psmm.tile([1, 4], F32, tag="mm")
            nc.tensor.matmul(pcol[:], ones_c[:], ptot[:], start=True, stop=True)
            nc.vector.tensor_copy(st_sb[:], pcol[:])
            nc.sync.dma_start(st_in[:], st_sb[:])
            if local_cc:
                nc.sync.dma_start(st_out.ap(), st_in.ap())
            else:
                nc.gpsimd.collective_compute(
                    "AllReduce", OP.add, replica_groups=[list(range(8))],
                    ins=[st_in.ap().opt()], outs=[st_out.ap().opt()])
            nc.sync.dma_start(st2_sb[:], st_out[:])
            bc = psmm.tile([128, 4], F32, tag="mm")
            nc.tensor.matmul(bc[:], ones_r[:], st2_sb[:], start=True, stop=True)
            nc.vector.tensor_copy(totals[:], bc[:])
            # s, thr, a  (all [128,4], replicated across partitions)
            nc.vector.tensor_tensor(s4[:], totals[:], inv_n[:], op=OP.mult)
            nc.vector.tensor_scalar(thr4[:], s4[:], EPS, ATANH05, op0=OP.add, op1=OP.mult)
            num = sm.tile([128, 4], F32, tag="num")
            den = sm.tile([128, 4], F32, tag="den")
            rat = sm.tile([128, 4], F32, tag="rat")
            nc.vector.tensor_scalar(num[:], s4[:], 1.0, None, op0=OP.add)
            nc.vector.tensor_scalar(den[:], s4[:], -1.0, 1.0, op0=OP.mult, op1=OP.add)
            nc.vector.reciprocal(rat[:], den[:])
            ratn = sm.tile([128, 4], F32, tag="ratn")
            nc.vector.tensor_tensor(ratn[:], den[:], rat[:], op=OP.mult)
            nc.vector.tensor_scalar(ratn[:], ratn[:], -1.0, 2.0, op0=OP.mult, op1=OP.add)
            nc.vector.tensor_tensor(rat[:], rat[:], ratn[:], op=OP.mult)
            nc.vector.tensor_tensor(rat[:], rat[:], num[:], op=OP.mult)
            lnr = sm.tile([128, 4], F32, tag="lnr")
            nc.scalar.activation(lnr[:], rat[:], AF.Ln)
            nc.vector.tensor_scalar(a4[:], lnr[:], 0.5, None, op0=OP.mult)
            nc.vector.tensor_scalar(aq128[:], a4[:, 0:1], 1.0 / 128.0, None, op0=OP.mult)
            hi4 = sm.tile([128, 4], F32, tag="hi4")
            nc.vector.reciprocal(hi4[:], thr4[:])
            hin = sm.tile([128, 4], F32, tag="hin")
            nc.vector.tensor_tensor(hin[:], thr4[:], hi4[:], op=OP.mult)
            nc.vector.tensor_scalar(hin[:], hin[:], -1.0, 2.0, op0=OP.mult, op1=OP.add)
            nc.vector.tensor_tensor(hi4[:], hi4[:], hin[:], op=OP.mult)
            nc.vector.tensor_scalar(hi4[:], hi4[:], 0.5, None, op0=OP.mult)

            # ---------- weights pass 2: ternary quantize ----------
            with tc.tile_pool(name="tern", bufs=2) as ternp:
                def ternary(src_ap, w, thr_col, out_ap):
                    # clip(round_half_even(w * 0.5/thr), -1, 1)
                    t = wstream.tile([128, w], F32, tag="wst")
                    nc.sync.dma_start(t[:, :w], src_ap)
                    u = ternp.tile([128, w], F32, tag="u", name="u")
                    nc.vector.tensor_scalar(u[:, :w], t[:, :w],
                                            hi4[:, thr_col:thr_col + 1], MAGIC,
                                            op0=OP.mult, op1=OP.add)
                    nc.vector.tensor_scalar(u[:, :w], u[:, :w], MAGIC, 1.0,
                                            op0=OP.subtract, op1=OP.min)
                    nc.vector.tensor_scalar(out_ap, u[:, :w], -1.0, None,
                                            op0=OP.max)

                for j in range(NB):
                    tq = ternp.tile([128, KVD], BF16, tag="tq")
                    ternary(wq_d[j * 128:(j + 1) * 128, :], KVD, 0, tq[:, :])
                    # group-sum 4 head blocks -> wqkv[:, 0:HD]
                    e1 = ternp.tile([128, HD], BF16, tag="e1")
                    e2 = ternp.tile([128, HD], BF16, tag="e2")
                    nc.vector.tensor_tensor(e1[:], tq[:, 0:HD], tq[:, HD:2 * HD], op=OP.add)
                    nc.vector.tensor_tensor(e2[:], tq[:, 2 * HD:3 * HD], tq[:, 3 * HD:4 * HD], op=OP.add)
                    nc.vector.tensor_tensor(wqkv_i[j][:, 0:HD], e1[:], e2[:], op=OP.add)
                    ternary(wk_d[j * 128:(j + 1) * 128, :], HD, 1, wqkv_i[j][:, HD:2 * HD])
                    ternary(wv_d[j * 128:(j + 1) * 128, :], HD, 2, wqkv_i[j][:, 2 * HD:3 * HD])
                for c in range(4):
                    ternary(wo_d[c * 128:(c + 1) * 128, :], D, 3, wo_i[c][:, :])

        # ---------- x phase: stats + int8 quantize + transpose ----------
        with tc.tile_pool(name="xqTp", bufs=1) as xqTp:
            xqT = xqTp.tile([128, NB, S], BF16, tag="xqT")
            with tc.tile_pool(name="xph", bufs=1) as xph:
                sq_scr = xph.tile([128, D], BF16, tag="sqscr")
                for i in range(NB):
                    xb = xph.tile([128, D], F32, tag="xb", bufs=2)
                    nc.sync.dma_start(xb[:], x_d[i * 128:(i + 1) * 128, :])
                    mx = xph.tile([128, 1], F32, tag="mx", bufs=2)
                    nc.vector.tensor_reduce(mx[:], xb[:], axis=AX.X, op=OP.max,
                                            apply_absolute_value=True)
                    ssq = xph.tile([128, 1], F32, tag="ssq", bufs=2)
                    nc.scalar.activation(sq_scr[:], xb[:], AF.Square, accum_out=ssq[:])
                    mean_t = xph.tile([128, 1], F32, tag="mean_t", bufs=2)
                    nc.vector.tensor_scalar(mean_t[:], ssq[:], 1.0 / D, EPS,
                                            op0=OP.mult, op1=OP.add)
                    sd = xph.tile([128, 1], F32, tag="sd", bufs=2)
                    nc.scalar.activation(sd[:], mean_t[:], AF.Sqrt)
                    r_ = xph.tile([128, 1], F32, tag="r", bufs=2)
                    nc.vector.reciprocal(r_[:], sd[:])
                    nt0 = xph.tile([128, 1], F32, tag="nt0", bufs=2)
                    nc.vector.tensor_tensor(nt0[:], r_[:], r_[:], op=OP.mult)
                    nc.vector.tensor_tensor(nt0[:], nt0[:], mean_t[:], op=OP.mult)
                    nc.vector.tensor_scalar(nt0[:], nt0[:], -0.5, 1.5, op0=OP.mult, op1=OP.add)
                    nc.vector.tensor_tensor(r_[:], r_[:], nt0[:], op=OP.mult)
                    m_ = xph.tile([128, 1], F32, tag="m", bufs=2)
                    nc.vector.tensor_tensor(m_[:], r_[:], mx[:], op=OP.mult)
                    nc.vector.tensor_scalar(m_[:], m_[:], 1e-4, None, op0=OP.max)
                    scl = xph.tile([128, 1], F32, tag="scl", bufs=2)
                    nc.vector.reciprocal(scl[:], m_[:])
                    nt1 = xph.tile([128, 1], F32, tag="nt1", bufs=2)
                    nc.vector.tensor_tensor(nt1[:], m_[:], scl[:], op=OP.mult)
                    nc.vector.tensor_scalar(nt1[:], nt1[:], -1.0, 2.0, op0=OP.mult, op1=OP.add)
                    nc.vector.tensor_tensor(scl[:], scl[:], nt1[:], op=OP.mult)
                    nc.vector.tensor_scalar(scl[:], scl[:], 127.0, None, op0=OP.mult)
                    nc.vector.reciprocal(deq_all[:, i:i + 1], scl[:])
                    nt2 = xph.tile([128, 1], F32, tag="nt2", bufs=2)
                    nc.vector.tensor_tensor(nt2[:], scl[:], deq_all[:, i:i + 1], op=OP.mult)
                    nc.vector.tensor_scalar(nt2[:], nt2[:], -1.0, 2.0, op0=OP.mult, op1=OP.add)
                    nc.vector.tensor_tensor(deq_all[:, i:i + 1], deq_all[:, i:i + 1], nt2[:], op=OP.mult)
                    smul = xph.tile([128, 1], F32, tag="smul", bufs=2)
                    nc.vector.tensor_tensor(smul[:], r_[:], scl[:], op=OP.mult)
                    # in-place: xb = xb*smul + MAGIC ; qb = xb - MAGIC (bf16)
                    nc.vector.tensor_scalar(xb[:], xb[:], smul[:], MAGIC,
                                            op0=OP.mult, op1=OP.add)
                    qb = xph.tile([128, D], BF16, tag="qb", bufs=2)
                    nc.scalar.activation(qb[:], xb[:], AF.Copy, bias=-MAGIC)
                    for jj in range(4):
                        tp = pstp.tile([128, 512], BF16, tag="tp")
                        for u in range(4):
                            j = 4 * jj + u
                            nc.tensor.transpose(tp[:, u * 128:(u + 1) * 128],
                                                qb[:, j * 128:(j + 1) * 128], idb[:])
                        dst = xqT[:, 4 * jj:4 * jj + 4, i * 128:(i + 1) * 128]
                        if jj % 2 == 0:
                            nc.vector.tensor_copy(dst, tp[:])
                        else:
                            nc.scalar.activation(dst, tp[:], AF.Copy)

            # ---------- QKV projections + dequant + rope + transpose ----------
            with tc.tile_pool(name="qkv", bufs=1) as qkv:
                v_all = qkv.tile([128, S], F32, tag="v_all")
                qT = qkv.tile([128, S], F32, tag="qT")
                kT = qkv.tile([128, S], F32, tag="kT")
                for i in range(NB):
                    pq = psmm.tile([128, 3 * HD], F32, tag="mm")
                    for j in range(NB):
                        nc.tensor.matmul(pq[:], xqT[:, j, i * 128:(i + 1) * 128],
                                         wqkv_i[j][:], start=(j == 0), stop=(j == NB - 1))
                    dq = qkv.tile([128, 1], F32, tag="dq", bufs=2)
                    dk = qkv.tile([128, 1], F32, tag="dk", bufs=2)
                    dv = qkv.tile([128, 1], F32, tag="dv", bufs=2)
                    nc.vector.tensor_tensor(dq[:], deq_all[:, i:i + 1], aq128[:], op=OP.mult)
                    nc.vector.tensor_tensor(dk[:], deq_all[:, i:i + 1], a4[:, 1:2], op=OP.mult)
                    nc.vector.tensor_tensor(dv[:], deq_all[:, i:i + 1], a4[:, 2:3], op=OP.mult)
                    qn = qkv.tile([128, HD], F32, tag="qn", bufs=2)
                    kn = qkv.tile([128, HD], F32, tag="kn", bufs=2)
                    nc.scalar.activation(qn[:], pq[:, 0:HD], AF.Copy, scale=dq[:])
                    nc.scalar.activation(kn[:], pq[:, HD:2 * HD], AF.Copy, scale=dk[:])
                    nc.scalar.activation(v_all[:, i * 128:(i + 1) * 128],
                                         pq[:, 2 * HD:3 * HD], AF.Copy, scale=dv[:])
                    # rope (even/odd halves contiguous by host weight permutation)
                    ci = cos_all[:, i, :]
                    si = sin_all[:, i, :]
                    hh = HD // 2
                    qr = qkv.tile([128, HD], F32, tag="qr", bufs=2)
                    kr = qkv.tile([128, HD], F32, tag="kr", bufs=2)
                    for src, dst in ((qn, qr), (kn, kr)):
                        t1 = qkv.tile([128, hh], F32, tag="rt1", bufs=2)
                        t2 = qkv.tile([128, hh], F32, tag="rt2", bufs=2)
                        nc.vector.tensor_tensor(t1[:], src[:, 0:hh], ci, op=OP.mult)
                        nc.vector.tensor_tensor(t2[:], src[:, hh:HD], si, op=OP.mult)
                        nc.vector.tensor_tensor(dst[:, 0:hh], t1[:], t2[:], op=OP.subtract)
                        t3 = qkv.tile([128, hh], F32, tag="rt3", bufs=2)
                        t4 = qkv.tile([128, hh], F32, tag="rt4", bufs=2)
                        nc.vector.tensor_tensor(t3[:], src[:, 0:hh], si, op=OP.mult)
                        nc.vector.tensor_tensor(t4[:], src[:, hh:HD], ci, op=OP.mult)
                        nc.vector.tensor_tensor(dst[:, hh:HD], t3[:], t4[:], op=OP.add)
                    tpq = pstp.tile([128, 512], F32, tag="tp")
                    nc.tensor.transpose(tpq[:, 0:128], qr[:], idf[:])
                    nc.tensor.transpose(tpq[:, 128:256], kr[:], idf[:])
                    nc.vector.tensor_copy(qT[:, i * 128:(i + 1) * 128], tpq[:, 0:128])
                    nc.scalar.activation(kT[:, i * 128:(i + 1) * 128], tpq[:, 128:256],
                                         AF.Copy)

                # ---------- attention ----------
                with tc.tile_pool(name="attn", bufs=1) as attn:
                    for i in range(NB):
                        nk = (i + 1) if causal else NB
                        nch = (nk * 128 + 511) // 512
                        S_sb = attn.tile([128, S], F32, tag="S", bufs=2)
                        zt = attn.tile([128, 4], F32, tag="zt", bufs=2)
                        for c in range(nch):
                            kw = min(512, nk * 128 - c * 512)
                            ps = psmm.tile([128, 512], F32, tag="mm")
                            nc.tensor.matmul(ps[:, :kw], qT[:, i * 128:(i + 1) * 128],
                                             kT[:, c * 512:c * 512 + kw],
                                             start=True, stop=True)
                            if causal and c == nch - 1:
                                nc.vector.tensor_tensor(ps[:, kw - 128:kw],
                                                        ps[:, kw - 128:kw], cmask[:],
                                                        op=OP.add)
                            if c % 2 == 0:
                                nc.vector.tensor_copy(S_sb[:, c * 512:c * 512 + kw],
                                                      ps[:, :kw])
                            else:
                                nc.scalar.activation(S_sb[:, c * 512:c * 512 + kw],
                                                     ps[:, :kw], AF.Copy)
                        mxs = attn.tile([128, 1], F32, tag="mxs", bufs=2)
                        nc.vector.tensor_reduce(mxs[:], S_sb[:, 0:nk * 128], axis=AX.X,
                                                op=OP.max)
                        ngm = attn.tile([128, 1], F32, tag="ngm", bufs=2)
                        nc.vector.tensor_scalar(ngm[:], mxs[:], -1.0, None, op0=OP.mult)
                        for c in range(nch):
                            kw = min(512, nk * 128 - c * 512)
                            nc.scalar.activation(S_sb[:, c * 512:c * 512 + kw],
                                                 S_sb[:, c * 512:c * 512 + kw],
                                                 AF.Exp, bias=ngm[:],
                                                 accum_out=zt[:, c:c + 1])
                        Zi = attn.tile([128, 1], F32, tag="Zi", bufs=2)
                        nc.vector.tensor_reduce(Zi[:], zt[:, 0:nch], axis=AX.X, op=OP.add)
                        rz = attn.tile([128, 1], F32, tag="rz", bufs=2)
                        nc.vector.reciprocal(rz[:], Zi[:])
                        PT = attn.tile([128, S], F32, tag="PT", bufs=2)
                        for kb4 in range((nk + 3) // 4):
                            nkb = min(4, nk - kb4 * 4)
                            tpP = pstp.tile([128, 512], F32, tag="tp")
                            for u in range(nkb):
                                kb = kb4 * 4 + u
                                nc.tensor.transpose(tpP[:, u * 128:(u + 1) * 128],
                                                    S_sb[:, kb * 128:(kb + 1) * 128],
                                                    idf[:])
                            dst = PT[:, kb4 * 512:kb4 * 512 + nkb * 128]
                            if kb4 % 2 == 0:
                                nc.vector.tensor_copy(dst, tpP[:, 0:nkb * 128])
                            else:
                                nc.scalar.activation(dst, tpP[:, 0:nkb * 128], AF.Copy)
                        po = psmm.tile([128, 512], F32, tag="mm")
                        for kb in range(nk):
                            nc.tensor.matmul(po[:, 0:HD], PT[:, kb * 128:(kb + 1) * 128],
                                             v_all[:, kb * 128:(kb + 1) * 128],
                                             start=(kb == 0), stop=(kb == nk - 1))
                        ob = attn.tile([128, HD], F32, tag="ob", bufs=2)
                        nc.scalar.activation(ob[:], po[:, 0:HD], AF.Copy, scale=rz[:])
                        qi = i // 4
                        ro = (i % 4) * 128
                        nc.sync.dma_start(cc_in[qi, ro:ro + 128, :], ob[:])
                        nc.sync.dma_start(cc_in[qi + 4, ro:ro + 128, :], ob[:])

        # ---------- exchange: padded 8-way AllToAll ----------
        if local_cc:
            nc.sync.dma_start(cc_out.ap(), cc_in.ap())
        else:
            nc.gpsimd.collective_compute(
                "AllToAll", OP.bypass, replica_groups=[list(range(8))],
                ins=[cc_in.ap().opt()], outs=[cc_out.ap().opt()])

        # ---------- output projection ----------
        with tc.tile_pool(name="outp", bufs=1) as outp:
            xoT = outp.tile([128, 4, SQ], BF16, tag="xoT")
            osc = outp.tile([128, KVD], BF16, tag="osc")
            qsel = cpool.tile([128, 2], F32, tag="qsel")
            nc.sync.dma_start(qsel[:], qsel_d[:])
            # Receive slots differ per quad (cores 0-3 read A2A slots 0-3, cores
            # 4-7 read slots 4-7) but the program is identical on every core: read
            # all 8 slots and select the right half with a per-core one-hot input.
            for tb in range(4):
                xo8 = outp.tile([128, 8 * HD], F32, tag="xo8", bufs=2)
                src = cc_out.ap()[:, tb * 128:(tb + 1) * 128, :].rearrange(
                    "j p d -> p j d")
                nc.sync.dma_start(xo8[:], src)
                xoa = outp.tile([128, KVD], F32, tag="xoa", bufs=2)
                nc.vector.tensor_scalar(xoa[:], xo8[:, 0:KVD], qsel[:, 0:1], None,
                                        op0=OP.mult)
                xo = outp.tile([128, KVD], F32, tag="xo", bufs=2)
                nc.vector.tensor_scalar(xo[:], xo8[:, KVD:2 * KVD], qsel[:, 1:2],
                                        None, op0=OP.mult)
                nc.vector.tensor_tensor(xo[:], xo[:], xoa[:], op=OP.add)
                mx2 = outp.tile([128, 1], F32, tag="mx2", bufs=2)
                nc.vector.tensor_reduce(mx2[:], xo[:], axis=AX.X, op=OP.max,
                                        apply_absolute_value=True)
                ssq2 = outp.tile([128, 1], F32, tag="ssq2", bufs=2)
                nc.scalar.activation(osc[:], xo[:], AF.Square, accum_out=ssq2[:])
                mean2 = outp.tile([128, 1], F32, tag="mean2", bufs=2)
                nc.vector.tensor_scalar(mean2[:], ssq2[:], 1.0 / KVD, EPS,
                                        op0=OP.mult, op1=OP.add)
                sd2 = outp.tile([128, 1], F32, tag="sd2", bufs=2)
                nc.scalar.activation(sd2[:], mean2[:], AF.Sqrt)
                r2 = outp.tile([128, 1], F32, tag="r2", bufs=2)
                nc.vector.reciprocal(r2[:], sd2[:])
                nt3 = outp.tile([128, 1], F32, tag="nt3", bufs=2)
                nc.vector.tensor_tensor(nt3[:], r2[:], r2[:], op=OP.mult)
                nc.vector.tensor_tensor(nt3[:], nt3[:], mean2[:], op=OP.mult)
                nc.vector.tensor_scalar(nt3[:], nt3[:], -0.5, 1.5, op0=OP.mult, op1=OP.add)
                nc.vector.tensor_tensor(r2[:], r2[:], nt3[:], op=OP.mult)
                m2 = outp.tile([128, 1], F32, tag="m2", bufs=2)
                nc.vector.tensor_tensor(m2[:], r2[:], mx2[:], op=OP.mult)
                nc.vector.tensor_scalar(m2[:], m2[:], 1e-4, None, op0=OP.max)
                scl2 = outp.tile([128, 1], F32, tag="scl2", bufs=2)
                nc.vector.reciprocal(scl2[:], m2[:])
                nt4 = outp.tile([128, 1], F32, tag="nt4", bufs=2)
                nc.vector.tensor_tensor(nt4[:], m2[:], scl2[:], op=OP.mult)
                nc.vector.tensor_scalar(nt4[:], nt4[:], -1.0, 2.0, op0=OP.mult, op1=OP.add)
                nc.vector.tensor_tensor(scl2[:], scl2[:], nt4[:], op=OP.mult)
                nc.vector.tensor_scalar(scl2[:], scl2[:], 127.0, None, op0=OP.mult)
                dqy = outp.tile([128, 1], F32, tag="dqy", bufs=2)
                nc.vector.reciprocal(dqy[:], scl2[:])
                nt5 = outp.tile([128, 1], F32, tag="nt5", bufs=2)
                nc.vector.tensor_tensor(nt5[:], scl2[:], dqy[:], op=OP.mult)
                nc.vector.tensor_scalar(nt5[:], nt5[:], -1.0, 2.0, op0=OP.mult, op1=OP.add)
                nc.vector.tensor_tensor(dqy[:], dqy[:], nt5[:], op=OP.mult)
                nc.vector.tensor_tensor(dqy[:], dqy[:], a4[:, 3:4], op=OP.mult)
                sm2 = outp.tile([128, 1], F32, tag="sm2", bufs=2)
                nc.vector.tensor_tensor(sm2[:], r2[:], scl2[:], op=OP.mult)
                nc.vector.tensor_scalar(xo[:], xo[:], sm2[:], MAGIC,
                                        op0=OP.mult, op1=OP.add)
                qo = outp.tile([128, KVD], BF16, tag="qo", bufs=2)
                nc.scalar.activation(qo[:], xo[:], AF.Copy, bias=-MAGIC)
                tpo = pstp.tile([128, 512], BF16, tag="tp")
                for jc in range(4):
                    nc.tensor.transpose(tpo[:, jc * 128:(jc + 1) * 128],
                                        qo[:, jc * 128:(jc + 1) * 128], idb[:])
                nc.vector.tensor_copy(xoT[:, 0:4, tb * 128:(tb + 1) * 128], tpo[:])
                y_sb = outp.tile([128, D], F32, tag="ysb", bufs=2)
                for oc in range(4):
                    py = psmm.tile([128, 512], F32, tag="mm")
                    for jc in range(4):
                        nc.tensor.matmul(py[:], xoT[:, jc, tb * 128:(tb + 1) * 128],
                                         wo_i[jc][:, oc * 512:(oc + 1) * 512],
                                         start=(jc == 0), stop=(jc == 3))
                    nc.scalar.activation(y_sb[:, oc * 512:(oc + 1) * 512], py[:],
                                         AF.Copy, scale=dqy[:])
                nc.sync.dma_start(y_d[tb * 128:(tb + 1) * 128, :], y_sb[:])
    nc.compile()
    return nc


def _rope_perm():
    p = np.empty(HD, np.int64)
    p[:HD // 2] = np.arange(0, HD, 2)
    p[HD // 2:] = np.arange(1, HD, 2)
    return p


def qsel_host(b):
    q = np.zeros((128, 2), np.float32)
    q[:, b] = 1.0
    return q


def _prep_inputs(inputs):
    x = np.ascontiguousarray(np.asarray(inputs["x"], np.float32))
    w_q = np.asarray(inputs["w_q"], np.float32)
    w_k = np.asarray(inputs["w_k"], np.float32)
    w_v = np.asarray(inputs["w_v"], np.float32)
    w_o = np.asarray(inputs["w_o"], np.float32)
    cos = np.ascontiguousarray(np.asarray(inputs["freq_cos"], np.float32))
    sin = np.ascontiguousarray(np.asarray(inputs["freq_sin"], np.float32))
    perm = _rope_perm()
    woT = np.ascontiguousarray(w_o.T)                      # [KVD, D]
    in_maps = []
    for r in range(8):
        b, kh = r // 4, r % 4
        heads = [g * KH + kh for g in range(4)]
        wq_sel = w_q.reshape(H, HD, D)[heads][:, perm, :]  # [4,128,D]
        wqT = np.ascontiguousarray(wq_sel.reshape(4 * HD, D).T)   # [D, 512]
        wkT = np.ascontiguousarray(w_k[kh * HD:(kh + 1) * HD][perm].T)  # [D,128]
        wvT = np.ascontiguousarray(w_v[kh * HD:(kh + 1) * HD].T)        # [D,128]
        in_maps.append({
            "x": x[b], "wq": wqT, "wk": wkT, "wv": wvT, "wo": woT,
            "cos": cos, "sin": sin,
            "qsel": qsel_host(b),
        })
    return in_maps


def _gains_trivial(inputs):
    return all(np.all(np.asarray(inputs[g]) == 1.0)
               for g in ("g_q", "g_k", "g_v", "g_o"))


def _numpy_fallback(inputs):
    """Faithful numpy reimplementation (slow); used only for unexpected configs."""
    x = np.asarray(inputs["x"], np.float32)
    cos, sin = (np.asarray(inputs[k], np.float32) for k in ("freq_cos", "freq_sin"))
    causal = int(np.asarray(inputs["causal"]))

    def rms(t, g):
        n = t * (1.0 / np.sqrt(np.mean(t * t, -1, keepdims=True, dtype=np.float32) + EPS))
        return (g * n).astype(np.float32)

    def actq(t):
        scale = 127.0 / np.clip(np.max(np.abs(t), -1, keepdims=True), 1e-4, None)
        q = np.round(t * scale)
        return np.clip(q, -128, 127) / scale

    def ternq(w):
        s = np.mean(np.abs(w), dtype=np.float32)
        return np.round(np.tanh(w / (s + EPS))) * np.arctanh(s)

    def lin(t, w, g):
        return actq(rms(t, g)).astype(np.float32) @ ternq(np.asarray(w, np.float32)).T

    Bb, Ss, Dd = x.shape
    q = lin(x, inputs["w_q"], np.asarray(inputs["g_q"], np.float32)).reshape(Bb, Ss, H, HD)
    k = lin(x, inputs["w_k"], np.asarray(inputs["g_k"], np.float32)).reshape(Bb, Ss, KH, HD)
    v = lin(x, inputs["w_v"], np.asarray(inputs["g_v"], np.float32)).reshape(Bb, Ss, KH, HD)

    def rope(t):
        t2 = t.reshape(*t.shape[:-1], -1, 2)
        c = cos[None, :, None, :]
        s_ = sin[None, :, None, :]
        o0 = t2[..., 0] * c - t2[..., 1] * s_
        o1 = t2[..., 0] * s_ + t2[..., 1] * c
        return np.stack([o0, o1], -1).reshape(t.shape).astype(np.float32)

    q, k = rope(q), rope(k)
    scale = np.float32(HD ** 0.5)
    q = q.transpose(0, 2, 1, 3) / scale
    k = k.transpose(0, 2, 1, 3)
    v = v.transpose(0, 2, 1, 3)
    qg = q.reshape(Bb, 4, KH, Ss, HD).sum(1)
    sc = np.einsum("bhnd,bhsd->bhns", qg, k).astype(np.float32)
    if causal:
        mask = np.tril(np.ones((Ss, Ss), bool))
        sc = np.where(mask[None, None], sc, np.float32(np.finfo(np.float32).min))
    sc = sc / scale
    sc = sc - sc.max(-1, keepdims=True)
    p = np.exp(sc)
    p /= p.sum(-1, keepdims=True)
    out = np.einsum("bhns,bhsd->bnhd", p, v).reshape(Bb, Ss, KVD)
    return lin(out, inputs["w_o"], np.asarray(inputs["g_o"], np.float32))


def kernel(**inputs):
    x = np.asarray(inputs["x"])
    if x.shape != (B, S, D) or not _gains_trivial(inputs):
        return _numpy_fallback(inputs)
    causal = bool(int(np.asarray(inputs["causal"])))
    key = ("bitattn", causal)
    if key not in _cache:
        _cache[key] = build(causal)
    nc = _cache[key]
    in_maps = _prep_inputs(inputs)
    res = run_bass_kernel_spmd(nc, in_maps, core_ids=list(range(8)))
    y = np.empty((B, S, D), np.float32)
    for r in range(8):
        b, qq = r // 4, r % 4
        y[b, qq * SQ:(qq + 1) * SQ, :] = res.results[r]["y"]
    return y


if __name__ == "__main__":
    data = np.load("/tmp/inputs.npz")
    inputs = {k: data[k] for k in data.files}
    out = kernel(**inputs)
    exp = np.load("/tmp/expected.npy")
    err = np.linalg.norm(out - exp) / np.linalg.norm(exp)
    print("Relative error:", err)



# revision 4
# speedup vs baseline: 1.4022x; 1.4022x over previous
"""BitAttention TRN2 kernel: 8-core SPMD (DP over batch x TP over kv-heads).

Self-contained: hardcodes shapes B=2, S=2048, D=2048, H=16, KH=4.
Core r: batch b = r//4, kv-head kh = r%4, stat/output stripe qq = r%4.

Math (forward-equivalent to the reference):
  - linear_bit = rms_norm -> per-row int8 act quant -> ternary weight quant
    -> matmul. Act-quant scale 127/max|xn| has rms self-cancel: the quantized
    ints are round(x*127/mx); rms enters only the per-token dequant scale.
  - Activations quantize straight to f16 with a +1536 offset (f16 ulp is 1 on
    [1024,2048) so the convert rounds half-to-even like jnp.round); the
    constant 1536 offset is removed inside the matmul by accumulating
    -1536*colsum(W) built from two exact f16 hi/lo matmuls.
  - Ternary weights quantize via round(clip(w*0.5/thr,-1,1)) (equivalent to
    round(tanh)), computed with the same +1536 f16 rounding trick.
  - The reference einsum sums the query-head group axis, so Q's 16 heads
    collapse to 4: group-sum the ternary w_q head blocks (ints in [-4,4]).
  - Scale folding: both 1/sqrt(HD) plus the wq/wk arctanh scales fold into
    the q rope tables (rope is linear); the wv scale cancels through the
    output rms-norm; the wo scale folds into the final dequant.
  - Attention runs transposed (S^T = K Q^T per key block) so softmaxed probs
    feed P^T directly into PV matmuls with no PE transposes; the softmax
    denominator comes from an extra all-ones column in the PV matmul.
    No max subtraction (logits are O(1) by construction).
  - Weight quantization work is sharded: batch-pair cores split w_q/w_k/w_v
    by row blocks, all 8 cores split w_o by columns; ternarized weights are
    exchanged with AllGather. Act-quant stats for x are sharded 4 ways across
    the batch group and AllGathered.
  - All tensor-engine transposes are done by DMA-transpose (f16/bf16).
"""
import numpy as np
from contextlib import ExitStack

import concourse.bass as bass
import concourse.bacc as bacc
import concourse.mybir as mybir
import concourse.tile as tile
from concourse.bass_utils import run_bass_kernel_spmd

B, S, D = 2, 2048, 2048
H, KH = 16, 4
HD = D // H          # 128
KVD = KH * HD        # 512
NB = S // 128        # 16 token blocks
SQ = S // 4          # 512 tokens per output stripe
EPS = 1e-8
MAGIC = float(1.5 * 2 ** 23)
M16 = 1536.0
ATANH05 = 0.5493061443340549      # arctanh(0.5)
NEG = -3.4e38
F32 = mybir.dt.float32
BF16 = mybir.dt.bfloat16
F16 = mybir.dt.float16
AX = mybir.AxisListType
OP = mybir.AluOpType
AF = mybir.ActivationFunctionType

_cache = {}


def _pt_off(kb, causal):
    if causal:
        return 2048 * kb - 64 * kb * (kb - 1)
    return 2048 * kb


def build(causal: bool, local_cc: bool = False):
    nc = bacc.Bacc()
    x_d = nc.dram_tensor("x", [S, D], F32, kind="ExternalInput")
    xs_d = nc.dram_tensor("xstat", [SQ, D], F32, kind="ExternalInput")
    wq_d = nc.dram_tensor("wq", [D // 2, KVD], F32, kind="ExternalInput")   # row-shard
    wk_d = nc.dram_tensor("wk", [D // 2, HD], F32, kind="ExternalInput")
    wv_d = nc.dram_tensor("wv", [D // 2, HD], F32, kind="ExternalInput")
    wo_d = nc.dram_tensor("wo", [KVD, D // 8], F32, kind="ExternalInput")   # col-shard
    cos_d = nc.dram_tensor("cos", [S, HD // 2], F32, kind="ExternalInput")
    sin_d = nc.dram_tensor("sin", [S, HD // 2], F32, kind="ExternalInput")
    qsel_d = nc.dram_tensor("qsel", [128, 2], F32, kind="ExternalInput")
    y_d = nc.dram_tensor("y", [SQ, D], F32, kind="ExternalOutput")
    st_in = nc.dram_tensor("st_in", [1, 4], F32)
    st_out = nc.dram_tensor("st_out", [1, 4], F32, addr_space="Shared")
    sc_in = nc.dram_tensor("sc_in", [4, 128, 2], F32)
    sc_out = nc.dram_tensor("sc_out", [4, 4, 128, 2], F32, addr_space="Shared")
    wg_in = nc.dram_tensor("wg_in", [8, 128, 3 * HD], F16)
    wg_out = nc.dram_tensor("wg_out", [2, 8, 128, 3 * HD], F16, addr_space="Shared")
    wob_in = nc.dram_tensor("wob_in", [4, 128, 256], BF16)
    wob_out = nc.dram_tensor("wob_out", [8, 4, 128, 256], BF16, addr_space="Shared")
    cc_in = nc.dram_tensor("cc_in", [4, 8, 128, HD], F32)
    cc_out = nc.dram_tensor("cc_out", [4, 8, 128, HD], F32, addr_space="Shared")

    PTW = _pt_off(NB, causal)

    with tile.TileContext(nc) as tc, ExitStack() as ctx:
        cpool = ctx.enter_context(tc.tile_pool(name="const", bufs=1))
        sm = ctx.enter_context(tc.tile_pool(name="sm", bufs=1))
        wres = ctx.enter_context(tc.tile_pool(name="wres", bufs=1))

        # ---------- constants ----------
        cmT = cpool.tile([128, 128], F32, tag="cmT")
        if causal:
            nc.gpsimd.memset(cmT[:], 0.0)
            nc.gpsimd.affine_select(out=cmT[:], in_=cmT[:], compare_op=OP.is_ge,
                                    fill=NEG, base=0, pattern=[[1, 128]],
                                    channel_multiplier=-1)
        ones_f16 = cpool.tile([128, 1], F16, tag="o16")
        nc.any.memset(ones_f16[:], 1.0)
        ones_bf = cpool.tile([128, 1], BF16, tag="obf")
        nc.any.memset(ones_bf[:], 1.0)
        ones_c = cpool.tile([128, 1], F32, tag="onc")
        nc.any.memset(ones_c[:], 1.0)
        ones_r = cpool.tile([1, 128], F32, tag="onr")
        nc.any.memset(ones_r[:], 1.0)
        n192 = cpool.tile([128, 128], F16, tag="n192")
        nc.any.memset(n192[:], -192.0)
        n12 = cpool.tile([128, 128], F16, tag="n12")
        nc.any.memset(n12[:], -12.0)
        inv_n = cpool.tile([128, 4], F32, tag="invn")
        for j, numel in enumerate([D * D, KVD * D, KVD * D, D * KVD]):
            nc.any.memset(inv_n[:, j:j + 1], 1.0 / numel)
        sqscr = cpool.tile([128, D], BF16, tag="sqscr")
        cos_kb = cpool.tile([128, NB, HD // 2], BF16, tag="coskb")
        sin_kb = cpool.tile([128, NB, HD // 2], BF16, tag="sinkb")
        cos_qb = cpool.tile([128, NB, HD // 2], BF16, tag="cosqb")
        sin_qb = cpool.tile([128, NB, HD // 2], BF16, tag="sinqb")

        # persistent smalls
        pr = sm.tile([128, 4], F32, tag="pr")
        st_sb = sm.tile([1, 4], F32, tag="st_sb")
        st2_sb = sm.tile([1, 4], F32, tag="st2_sb")
        totals = sm.tile([128, 4], F32, tag="totals")
        s4 = sm.tile([128, 4], F32, tag="s4")
        hi4 = sm.tile([128, 4], F32, tag="hi4")
        a4 = sm.tile([128, 4], F32, tag="a4")
        aqk = sm.tile([128, 1], F32, tag="aqk")
        mxs = sm.tile([128, 4], F32, tag="mxs")
        ssqs = sm.tile([128, 4], F32, tag="ssqs")
        sx_sb = sm.tile([128, 4, 2], F32, tag="sx_sb")
        sd_all = sm.tile([128, 16, 2], F32, tag="sd_all")
        csr = sm.tile([1, 3 * HD], F32, tag="csr")
        csbc = sm.tile([128, 3 * HD], F32, tag="csbc")
        hq = sm.tile([128, 3 * HD], F16, tag="hq")
        lq = sm.tile([128, 3 * HD], F16, tag="lq")

        # persistent quantized weights
        wqkv = wres.tile([128, NB, 3 * HD], F16, tag="wqkv", name="wqkv")
        wo_q = wres.tile([128, 4, D], BF16, tag="wo_q", name="wo_q")

        # cos/sin staging (f32, freed after table builds)
        with tc.tile_pool(name="cstage", bufs=1) as cstage:
            cosf = cstage.tile([128, NB, HD // 2], F32, tag="cosf")
            sinf = cstage.tile([128, NB, HD // 2], F32, tag="sinf")
            nc.sync.dma_start(cosf[:], cos_d.ap().rearrange("(i p) f -> p i f", p=128))
            nc.sync.dma_start(sinf[:], sin_d.ap().rearrange("(i p) f -> p i f", p=128))
            nc.vector.tensor_copy(cos_kb[:], cosf[:])
            nc.vector.tensor_copy(sin_kb[:], sinf[:])

            # ---------- weight phase (sharded) ----------
            with tc.tile_pool(name="wph", bufs=1) as wph:
                wq_sb = wph.tile([128, 8, KVD], F32, tag="wq_sb")
                wk_sb = wph.tile([128, 8, HD], F32, tag="wk_sb")
                wv_sb = wph.tile([128, 8, HD], F32, tag="wv_sb")
                wo_sb = wph.tile([128, 4, 256], F32, tag="wo_sb")
                nc.sync.dma_start(wq_sb[:], wq_d.ap().rearrange("(j p) c -> p j c", p=128))
                nc.sync.dma_start(wk_sb[:], wk_d.ap().rearrange("(j p) c -> p j c", p=128))
                nc.sync.dma_start(wv_sb[:], wv_d.ap().rearrange("(j p) c -> p j c", p=128))
                nc.sync.dma_start(wo_sb[:], wo_d.ap().rearrange("(c p) d -> p c d", p=128))

                # pass1: |w| row sums (each element once across the fleet)
                nc.vector.tensor_reduce(pr[:, 0:1], wq_sb[:].rearrange("p a b -> p (a b)"),
                                        axis=AX.X, op=OP.add, apply_absolute_value=True)
                nc.vector.tensor_reduce(pr[:, 1:2], wk_sb[:].rearrange("p a b -> p (a b)"),
                                        axis=AX.X, op=OP.add, apply_absolute_value=True)
                nc.vector.tensor_reduce(pr[:, 2:3], wv_sb[:].rearrange("p a b -> p (a b)"),
                                        axis=AX.X, op=OP.add, apply_absolute_value=True)
                nc.vector.tensor_reduce(pr[:, 3:4], wo_sb[:].rearrange("p a b -> p (a b)"),
                                        axis=AX.X, op=OP.add, apply_absolute_value=True)

                with tc.tile_pool(name="psst", bufs=2, space="PSUM") as psst:
                    pcol = psst.tile([1, 4], F32, tag="st")
                    nc.tensor.matmul(pcol[:], ones_c[:], pr[:], start=True, stop=True)
                    nc.vector.tensor_copy(st_sb[:], pcol[:])
                    nc.sync.dma_start(st_in[:], st_sb[:])
                    if local_cc:
                        nc.sync.dma_start(st_out.ap(), st_in.ap())
                    else:
                        nc.gpsimd.collective_compute(
                            "AllReduce", OP.add, replica_groups=[list(range(8))],
                            ins=[st_in.ap().opt()], outs=[st_out.ap().opt()])
                    nc.sync.dma_start(st2_sb[:], st_out[:])
                    bc = psst.tile([128, 4], F32, tag="st")
                    nc.tensor.matmul(bc[:], ones_r[:], st2_sb[:], start=True, stop=True)
                    nc.vector.tensor_copy(totals[:], bc[:])

                # s, 1/(2 thr), a  (all [128,4] replicated)
                nc.vector.tensor_tensor(s4[:], totals[:], inv_n[:], op=OP.mult)
                thr2 = sm.tile([128, 4], F32, tag="thr2")
                nc.vector.tensor_scalar(thr2[:], s4[:], EPS, 2.0 * ATANH05,
                                        op0=OP.add, op1=OP.mult)
                nc.vector.reciprocal(hi4[:], thr2[:])
                ntp = sm.tile([128, 4], F32, tag="ntp")
                nc.vector.tensor_tensor(ntp[:], thr2[:], hi4[:], op=OP.mult)
                nc.vector.tensor_scalar(ntp[:], ntp[:], -1.0, 2.0, op0=OP.mult, op1=OP.add)
                nc.vector.tensor_tensor(hi4[:], hi4[:], ntp[:], op=OP.mult)
                num = sm.tile([128, 4], F32, tag="num")
                den = sm.tile([128, 4], F32, tag="den")
                rat = sm.tile([128, 4], F32, tag="rat")
                nc.vector.tensor_scalar(num[:], s4[:], 1.0, None, op0=OP.add)
                nc.vector.tensor_scalar(den[:], s4[:], -1.0, 1.0, op0=OP.mult, op1=OP.add)
                nc.vector.reciprocal(rat[:], den[:])
                nc.vector.tensor_tensor(ntp[:], den[:], rat[:], op=OP.mult)
                nc.vector.tensor_scalar(ntp[:], ntp[:], -1.0, 2.0, op0=OP.mult, op1=OP.add)
                nc.vector.tensor_tensor(rat[:], rat[:], ntp[:], op=OP.mult)
                nc.vector.tensor_tensor(rat[:], rat[:], num[:], op=OP.mult)
                lnr = sm.tile([128, 4], F32, tag="lnr")
                nc.scalar.activation(lnr[:], rat[:], AF.Ln)
                nc.vector.tensor_scalar(a4[:], lnr[:], 0.5, None, op0=OP.mult)
                nc.vector.tensor_tensor(aqk[:], a4[:, 0:1], a4[:, 1:2], op=OP.mult)
                nc.vector.tensor_scalar(aqk[:], aqk[:], 1.0 / HD, None, op0=OP.mult)
                # scaled q rope tables (fold a_q*a_k/HD into q's rotation)
                nc.vector.tensor_scalar(cos_qb[:], cosf[:], aqk[:], None, op0=OP.mult)
                nc.vector.tensor_scalar(sin_qb[:], sinf[:], aqk[:], None, op0=OP.mult)

                # ternary quantize shards.  passA (pool): u = min(w*hi, 1)
                # passB (DVE): f16(max(u,-1) + 1536); passC: sub 1536
                wsh = wph.tile([128, 8, 3 * HD], F16, tag="wsh")
                wosh = wph.tile([128, 4, 256], BF16, tag="wosh")
                tq = wph.tile([128, 8, KVD], F16, tag="tq")
                tk = wph.tile([128, 8, HD], F16, tag="tk")
                tg1 = wph.tile([128, 8, HD], F16, tag="tg1")
                tg2 = wph.tile([128, 8, HD], F16, tag="tg2")

                def ternA(t, col):
                    nc.gpsimd.tensor_scalar(t[:], t[:], hi4[:, col:col + 1], 1.0,
                                            op0=OP.mult, op1=OP.min)

                ternA(wq_sb, 0)
                nc.vector.tensor_scalar(tq[:], wq_sb[:], -1.0, M16, op0=OP.max, op1=OP.add)
                nc.vector.tensor_scalar(tq[:], tq[:], M16, None, op0=OP.subtract)
                # group-sum 4 head blocks (ints in [-4,4], exact f16)
                nc.vector.tensor_tensor(tg1[:], tq[:, :, 0:HD], tq[:, :, HD:2 * HD], op=OP.add)
                nc.vector.tensor_tensor(tg2[:], tq[:, :, 2 * HD:3 * HD], tq[:, :, 3 * HD:4 * HD], op=OP.add)
                nc.vector.tensor_tensor(wsh[:, :, 0:HD], tg1[:], tg2[:], op=OP.add)
                ternA(wk_sb, 1)
                nc.vector.tensor_scalar(tk[:], wk_sb[:], -1.0, M16, op0=OP.max, op1=OP.add)
                nc.vector.tensor_scalar(wsh[:, :, HD:2 * HD], tk[:], M16, None, op0=OP.subtract)
                ternA(wv_sb, 2)
                nc.vector.tensor_scalar(tk[:], wv_sb[:], -1.0, M16, op0=OP.max, op1=OP.add)
                nc.vector.tensor_scalar(wsh[:, :, 2 * HD:3 * HD], tk[:], M16, None, op0=OP.subtract)
                ternA(wo_sb, 3)
                to16 = wph.tile([128, 4, 256], F16, tag="to16")
                nc.vector.tensor_scalar(to16[:], wo_sb[:], -1.0, M16, op0=OP.max, op1=OP.add)
                nc.vector.tensor_scalar(wosh[:], to16[:], M16, None, op0=OP.subtract)

                # exchange ternary shards (ACT-issued DMAs to keep SP free)
                nc.scalar.dma_start(wg_in.ap().rearrange("j p c -> p j c"), wsh[:])
                nc.scalar.dma_start(wob_in.ap().rearrange("c p d -> p c d"), wosh[:])
                if local_cc:
                    nc.scalar.dma_start(wg_out.ap()[0], wg_in.ap())
                    nc.scalar.dma_start(wob_out.ap()[0], wob_in.ap())
                else:
                    nc.gpsimd.collective_compute(
                        "AllGather", OP.bypass,
                        replica_groups=[[0, 4], [1, 5], [2, 6], [3, 7]],
                        ins=[wg_in.ap().opt()], outs=[wg_out.ap().opt()])
                    nc.gpsimd.collective_compute(
                        "AllGather", OP.bypass, replica_groups=[list(range(8))],
                        ins=[wob_in.ap().opt()], outs=[wob_out.ap().opt()])
                for s in range(2):
                    nc.scalar.dma_start(
                        wqkv[:].rearrange("p (j s) c -> p j s c", s=2)[:, :, s, :],
                        wg_out.ap()[s].rearrange("j p c -> p j c"))
                for s in range(8):
                    nc.scalar.dma_start(
                        wo_q[:].rearrange("p c (s d) -> p c s d", s=8)[:, :, s, :],
                        wob_out.ap()[s].rearrange("c p d -> p c d"))

        # ---------- x stats (sharded 4-way over the batch group) ----------
        with tc.tile_pool(name="xph", bufs=1) as xph:
            for i in range(4):
                xs = xph.tile([128, D], F32, tag="xb", bufs=2)
                nc.sync.dma_start(xs[:], xs_d[i * 128:(i + 1) * 128, :])
                nc.vector.tensor_reduce(mxs[:, i:i + 1], xs[:], axis=AX.X,
                                        op=OP.max, apply_absolute_value=True)
                nc.scalar.activation(sqscr[:], xs[:], AF.Square,
                                     accum_out=ssqs[:, i:i + 1])
            mean4 = sm.tile([128, 4], F32, tag="mean4")
            nc.vector.tensor_scalar(mean4[:], ssqs[:], 1.0 / D, EPS, op0=OP.mult, op1=OP.add)
            sd4 = sm.tile([128, 4], F32, tag="sd4")
            nc.scalar.activation(sd4[:], mean4[:], AF.Sqrt)
            r4 = sm.tile([128, 4], F32, tag="r4")
            nc.vector.reciprocal(r4[:], sd4[:])
            nt4 = sm.tile([128, 4], F32, tag="nt4")
            nc.vector.tensor_tensor(nt4[:], sd4[:], r4[:], op=OP.mult)
            nc.vector.tensor_scalar(nt4[:], nt4[:], -1.0, 2.0, op0=OP.mult, op1=OP.add)
            nc.vector.tensor_tensor(r4[:], r4[:], nt4[:], op=OP.mult)
            m127 = sm.tile([128, 4], F32, tag="m127")
            nc.vector.tensor_scalar(m127[:], mxs[:], 1.0 / 127.0, None, op0=OP.mult)
            smul4 = sm.tile([128, 4], F32, tag="smul4")
            nc.vector.reciprocal(smul4[:], m127[:])
            nc.vector.tensor_tensor(nt4[:], m127[:], smul4[:], op=OP.mult)
            nc.vector.tensor_scalar(nt4[:], nt4[:], -1.0, 2.0, op0=OP.mult, op1=OP.add)
            nc.vector.tensor_tensor(smul4[:], smul4[:], nt4[:], op=OP.mult)
            deq4 = sm.tile([128, 4], F32, tag="deq4")
            nc.vector.tensor_tensor(deq4[:], mxs[:], r4[:], op=OP.mult)
            nc.vector.tensor_scalar(deq4[:], deq4[:], 1.0 / 127.0, None, op0=OP.mult)
            nc.vector.tensor_copy(sx_sb[:, :, 0], smul4[:])
            nc.vector.tensor_copy(sx_sb[:, :, 1], deq4[:])
            nc.sync.dma_start(sc_in.ap().rearrange("i p c -> p i c"), sx_sb[:])
            if local_cc:
                nc.sync.dma_start(sc_out.ap()[0], sc_in.ap())
            else:
                nc.gpsimd.collective_compute(
                    "AllGather", OP.bypass,
                    replica_groups=[[0, 1, 2, 3], [4, 5, 6, 7]],
                    ins=[sc_in.ap().opt()], outs=[sc_out.ap().opt()])
            nc.sync.dma_start(sd_all[:], sc_out.ap().rearrange("s i p c -> p (s i) c"))

            # ---------- x quantize + transpose ----------
            with tc.tile_pool(name="xqTp", bufs=1) as xqTp:
                xqT = xqTp.tile([128, NB, S], F16, tag="xqT")
                for i in range(NB):
                    xb = xph.tile([128, D], F32, tag="xb", bufs=2)
                    nc.sync.dma_start(xb[:], x_d[i * 128:(i + 1) * 128, :])
                    qb = xph.tile([128, D], F16, tag="qb", bufs=3)
                    if i % 2 == 0:
                        nc.vector.tensor_scalar(qb[:], xb[:], sd_all[:, i, 0:1], M16,
                                                op0=OP.mult, op1=OP.add)
                    else:
                        nc.scalar.activation(qb[:], xb[:], AF.Copy, bias=M16,
                                             scale=sd_all[:, i, 0:1])
                    nc.sync.dma_start_transpose(xqT[:, :, i * 128:(i + 1) * 128], qb[:])

                # csum of wqkv (for the -1536 offset correction), f16 hi/lo split
                qkv_all = sm.tile([128, NB, 3 * HD], BF16, tag="qkv_all", name="qkv_all")
                with tc.tile_pool(name="psc", bufs=2, space="PSUM") as psc:
                    csp = psc.tile([1, 3 * HD], F32, tag="cs")
                    for j in range(NB):
                        nc.tensor.matmul(csp[:], ones_f16[:], wqkv[:, j, :],
                                         start=(j == 0), stop=(j == NB - 1))
                    nc.vector.tensor_copy(csr[:], csp[:])
                    bcp = psc.tile([128, 3 * HD], F32, tag="cs")
                    nc.tensor.matmul(bcp[:], ones_r[:], csr[:], start=True, stop=True)
                    nc.vector.tensor_copy(csbc[:], bcp[:])
                    tcs = sm.tile([128, 3 * HD], F32, tag="tcs")
                    nc.vector.tensor_scalar(tcs[:], csbc[:], 1.0 / 16.0, MAGIC,
                                            op0=OP.mult, op1=OP.add)
                    nc.vector.tensor_scalar(tcs[:], tcs[:], MAGIC, None, op0=OP.subtract)
                    nc.vector.tensor_copy(hq[:], tcs[:])
                    nc.vector.scalar_tensor_tensor(lq[:], tcs[:], -16.0, csbc[:],
                                                   op0=OP.mult, op1=OP.add)

                    # ---------- QKV projections ----------
                    for i in range(NB):
                        pq = psc.tile([128, 3 * HD], F32, tag="mm")
                        for j in range(NB):
                            nc.tensor.matmul(pq[:], xqT[:, j, i * 128:(i + 1) * 128],
                                             wqkv[:, j, :], start=(j == 0), stop=False)
                        nc.tensor.matmul(pq[:], n192[:], hq[:], start=False, stop=False,
                                         skip_group_check=True)
                        nc.tensor.matmul(pq[:], n12[:], lq[:], start=False, stop=True,
                                         skip_group_check=True)
                        nc.scalar.activation(qkv_all[:, i, :], pq[:], AF.Copy,
                                             scale=sd_all[:, i, 1:2])

        # ---------- rope (batched, bf16) + transposes ----------
        atl = ctx.enter_context(tc.tile_pool(name="atl", bufs=1))
        qT = atl.tile([128, NB, 128], BF16, tag="qT", name="qT")
        kT = atl.tile([128, NB, 128], BF16, tag="kT", name="kT")
        with tc.tile_pool(name="rp", bufs=1) as rp:
            hh = HD // 2
            qr = rp.tile([128, NB, HD], BF16, tag="qr")
            kr = rp.tile([128, NB, HD], BF16, tag="kr")
            for src0, cosb, sinb, dst in ((0, cos_qb, sin_qb, qr),
                                          (HD, cos_kb, sin_kb, kr)):
                ev = qkv_all[:, :, src0:src0 + hh]
                od = qkv_all[:, :, src0 + hh:src0 + HD]
                t1 = rp.tile([128, NB, hh], BF16, tag="t1", bufs=2)
                t2 = rp.tile([128, NB, hh], BF16, tag="t2", bufs=2)
                nc.vector.tensor_tensor(t1[:], ev, cosb[:], op=OP.mult)
                nc.vector.tensor_tensor(t2[:], od, sinb[:], op=OP.mult)
                nc.vector.tensor_tensor(dst[:, :, 0:hh], t1[:], t2[:], op=OP.subtract)
                t3 = rp.tile([128, NB, hh], BF16, tag="t1", bufs=2)
                t4 = rp.tile([128, NB, hh], BF16, tag="t2", bufs=2)
                nc.vector.tensor_tensor(t3[:], ev, sinb[:], op=OP.mult)
                nc.vector.tensor_tensor(t4[:], od, cosb[:], op=OP.mult)
                nc.vector.tensor_tensor(dst[:, :, hh:HD], t3[:], t4[:], op=OP.add)
            nc.sync.dma_start_transpose(qT[:], qr[:])
            nc.sync.dma_start_transpose(kT[:], kr[:])

        # ---------- attention + output projection ----------
        qTf = qT[:].rearrange("p a b -> p (a b)")
        with tc.tile_pool(name="attn", bufs=1) as attn, \
             tc.tile_pool(name="pss", bufs=3, space="PSUM") as pss, \
             tc.tile_pool(name="psv", bufs=2, space="PSUM") as psv, \
             tc.tile_pool(name="psy", bufs=2, space="PSUM") as psy:
            PT = attn.tile([128, PTW], BF16, tag="PT")
            obuf = attn.tile([128, 4, HD], F32, tag="obuf", bufs=2)

            def scores(kb):
                qlo = 128 * kb if causal else 0
                c0 = qlo
                first = True
                while c0 < S:
                    cw = min(512, S - c0)
                    sp = pss.tile([128, 512], F32, tag="sc")
                    nc.tensor.matmul(sp[:, 0:cw], kT[:, kb, :], qTf[:, c0:c0 + cw],
                                     start=True, stop=True)
                    if causal and first:
                        nc.vector.tensor_tensor(sp[:, 0:128], sp[:, 0:128], cmT[:],
                                                op=OP.add)
                    nc.scalar.activation(PT[:, _pt_off(kb, causal) + c0 - qlo:
                                            _pt_off(kb, causal) + c0 - qlo + cw],
                                         sp[:, 0:cw], AF.Exp)
                    first = False
                    c0 += cw

            def pv(qb):
                po = psv.tile([128, 132], F32, tag="po")
                nkb = qb + 1 if causal else NB
                for k2 in range(nkb):
                    qoff = (qb - k2) * 128 if causal else qb * 128
                    lhs = PT[:, _pt_off(k2, causal) + qoff:
                             _pt_off(k2, causal) + qoff + 128]
                    nc.tensor.matmul(po[:, 0:HD], lhs, qkv_all[:, k2, 2 * HD:3 * HD],
                                     start=(k2 == 0), stop=(k2 == nkb - 1),
                                     skip_group_check=True)
                    nc.tensor.matmul(po[:, HD:HD + 1], lhs, ones_bf[:],
                                     start=(k2 == 0), stop=(k2 == nkb - 1),
                                     skip_group_check=True)
                rz = attn.tile([128, 1], F32, tag="rz", bufs=2)
                nz = attn.tile([128, 1], F32, tag="nz", bufs=2)
                nc.vector.reciprocal(rz[:], po[:, HD:HD + 1])
                nc.vector.tensor_tensor(nz[:], po[:, HD:HD + 1], rz[:], op=OP.mult)
                nc.vector.tensor_scalar(nz[:], nz[:], -1.0, 2.0, op0=OP.mult, op1=OP.add)
                nc.vector.tensor_tensor(rz[:], rz[:], nz[:], op=OP.mult)
                nc.scalar.activation(obuf[:, qb % 4, :], po[:, 0:HD], AF.Copy,
                                     scale=rz[:])

            def oproj(tb):
                xo8 = attn.tile([128, 8, HD], F32, tag="xo8", bufs=2)
                nc.sync.dma_start(xo8[:], cc_out.ap()[tb].rearrange("s p d -> p s d"))
                xsel = attn.tile([128, KVD], F32, tag="xsel", bufs=2)
                nc.gpsimd.tensor_scalar(xsel[:], xo8[:, 0:4, :].rearrange("p a b -> p (a b)"),
                                        qsel[:, 0:1], None, op0=OP.mult)
                xo = attn.tile([128, KVD], F32, tag="xo", bufs=2)
                nc.gpsimd.scalar_tensor_tensor(xo[:], xo8[:, 4:8, :].rearrange("p a b -> p (a b)"),
                                               qsel[:, 1:2], xsel[:],
                                               op0=OP.mult, op1=OP.add)
                mx2 = attn.tile([128, 1], F32, tag="mx2", bufs=2)
                nc.vector.tensor_reduce(mx2[:], xo[:], axis=AX.X, op=OP.max,
                                        apply_absolute_value=True)
                ssq2 = attn.tile([128, 1], F32, tag="ssq2", bufs=2)
                nc.scalar.activation(sqscr[:, 0:KVD], xo[:], AF.Square, accum_out=ssq2[:])
                mean2 = attn.tile([128, 1], F32, tag="mean2", bufs=2)
                nc.vector.tensor_scalar(mean2[:], ssq2[:], 1.0 / KVD, EPS,
                                        op0=OP.mult, op1=OP.add)
                sd2 = attn.tile([128, 1], F32, tag="sd2", bufs=2)
                nc.scalar.activation(sd2[:], mean2[:], AF.Sqrt)
                r2 = attn.tile([128, 1], F32, tag="r2", bufs=2)
                nt2 = attn.tile([128, 1], F32, tag="nt2", bufs=2)
                nc.vector.reciprocal(r2[:], sd2[:])
                nc.vector.tensor_tensor(nt2[:], sd2[:], r2[:], op=OP.mult)
                nc.vector.tensor_scalar(nt2[:], nt2[:], -1.0, 2.0, op0=OP.mult, op1=OP.add)
                nc.vector.tensor_tensor(r2[:], r2[:], nt2[:], op=OP.mult)
                m2 = attn.tile([128, 1], F32, tag="m2", bufs=2)
                nc.vector.tensor_scalar(m2[:], mx2[:], 1.0 / 127.0, None, op0=OP.mult)
                sl2 = attn.tile([128, 1], F32, tag="sl2", bufs=2)
                nc.vector.reciprocal(sl2[:], m2[:])
                nc.vector.tensor_tensor(nt2[:], m2[:], sl2[:], op=OP.mult)
                nc.vector.tensor_scalar(nt2[:], nt2[:], -1.0, 2.0, op0=OP.mult, op1=OP.add)
                nc.vector.tensor_tensor(sl2[:], sl2[:], nt2[:], op=OP.mult)
                dqy = attn.tile([128, 1], F32, tag="dqy", bufs=2)
                nc.vector.tensor_tensor(dqy[:], mx2[:], r2[:], op=OP.mult)
                nc.vector.tensor_scalar(dqy[:], dqy[:], 1.0 / 127.0, None, op0=OP.mult)
                nc.vector.tensor_tensor(dqy[:], dqy[:], a4[:, 3:4], op=OP.mult)
                nc.vector.tensor_scalar(xo[:], xo[:], sl2[:], MAGIC,
                                        op0=OP.mult, op1=OP.add)
                qo = attn.tile([128, KVD], BF16, tag="qo", bufs=2)
                nc.scalar.activation(qo[:], xo[:], AF.Copy, bias=-MAGIC)
                xoT = attn.tile([128, 4, 128], BF16, tag="xoT", bufs=2)
                nc.sync.dma_start_transpose(xoT[:], qo[:])
                y_sb = attn.tile([128, D], F32, tag="ysb", bufs=2)
                for oc in range(4):
                    py = psy.tile([128, 512], F32, tag="my")
                    for jc in range(4):
                        nc.tensor.matmul(py[:], xoT[:, jc, :],
                                         wo_q[:, jc, oc * 512:(oc + 1) * 512],
                                         start=(jc == 0), stop=(jc == 3))
                    if oc % 2 == 0:
                        nc.scalar.activation(y_sb[:, oc * 512:(oc + 1) * 512], py[:],
                                             AF.Copy, scale=dqy[:])
                    else:
                        nc.vector.tensor_scalar(y_sb[:, oc * 512:(oc + 1) * 512],
                                                py[:], dqy[:], None, op0=OP.mult)
                nc.sync.dma_start(y_d[tb * 128:(tb + 1) * 128, :], y_sb[:])

            qsel = cpool.tile([128, 2], F32, tag="qsel")
            nc.sync.dma_start(qsel[:], qsel_d[:])
            if causal:
                for kb in range(NB):
                    scores(kb)
                    pv(kb)
                    if kb % 4 == 3:
                        r4i = kb // 4
                        nc.sync.dma_start(cc_in.ap()[r4i, 0:4].rearrange("s p d -> p s d"),
                                          obuf[:])
                        nc.sync.dma_start(cc_in.ap()[r4i, 4:8].rearrange("s p d -> p s d"),
                                          obuf[:])
                        if local_cc:
                            nc.sync.dma_start(cc_out.ap()[r4i], cc_in.ap()[r4i])
                        else:
                            nc.gpsimd.collective_compute(
                                "AllToAll", OP.bypass, replica_groups=[list(range(8))],
                                ins=[cc_in.ap()[r4i].opt()], outs=[cc_out.ap()[r4i].opt()])
                        oproj(r4i)
            else:
                for kb in range(NB):
                    scores(kb)
                for qb in range(NB):
                    pv(qb)
                    if qb % 4 == 3:
                        r4i = qb // 4
                        nc.sync.dma_start(cc_in.ap()[r4i, 0:4].rearrange("s p d -> p s d"),
                                          obuf[:])
                        nc.sync.dma_start(cc_in.ap()[r4i, 4:8].rearrange("s p d -> p s d"),
                                          obuf[:])
                        if local_cc:
                            nc.sync.dma_start(cc_out.ap()[r4i], cc_in.ap()[r4i])
                        else:
                            nc.gpsimd.collective_compute(
                                "AllToAll", OP.bypass, replica_groups=[list(range(8))],
                                ins=[cc_in.ap()[r4i].opt()], outs=[cc_out.ap()[r4i].opt()])
                        oproj(r4i)
    nc.compile()
    return nc


def _rope_perm():
    p = np.empty(HD, np.int64)
    p[:HD // 2] = np.arange(0, HD, 2)
    p[HD // 2:] = np.arange(1, HD, 2)
    return p


def qsel_host(b):
    q = np.zeros((128, 2), np.float32)
    q[:, b] = 1.0
    return q


def _prep_inputs(inputs):
    x = np.ascontiguousarray(np.asarray(inputs["x"], np.float32))
    w_q = np.asarray(inputs["w_q"], np.float32)
    w_k = np.asarray(inputs["w_k"], np.float32)
    w_v = np.asarray(inputs["w_v"], np.float32)
    w_o = np.asarray(inputs["w_o"], np.float32)
    cos = np.ascontiguousarray(np.asarray(inputs["freq_cos"], np.float32))
    sin = np.ascontiguousarray(np.asarray(inputs["freq_sin"], np.float32))
    perm = _rope_perm()
    woT = np.ascontiguousarray(w_o.T)                      # [KVD, D]
    in_maps = []
    jrows = np.arange(D) // 128 % 2
    for r in range(8):
        b, kh = r // 4, r % 4
        heads = [g * KH + kh for g in range(4)]
        wq_sel = w_q.reshape(H, HD, D)[heads][:, perm, :]  # [4,128,D]
        wqT = np.ascontiguousarray(wq_sel.reshape(4 * HD, D).T)   # [D, 512]
        wkT = np.ascontiguousarray(w_k[kh * HD:(kh + 1) * HD][perm].T)  # [D,128]
        wvT = np.ascontiguousarray(w_v[kh * HD:(kh + 1) * HD].T)        # [D,128]
        sel = jrows == b
        in_maps.append({
            "x": x[b],
            "xstat": np.ascontiguousarray(x[b][kh * SQ:(kh + 1) * SQ]),
            "wq": np.ascontiguousarray(wqT[sel]),
            "wk": np.ascontiguousarray(wkT[sel]),
            "wv": np.ascontiguousarray(wvT[sel]),
            "wo": np.ascontiguousarray(woT[:, r * 256:(r + 1) * 256]),
            "cos": cos, "sin": sin,
            "qsel": qsel_host(b),
        })
    return in_maps


def _gains_trivial(inputs):
    return all(np.all(np.asarray(inputs[g]) == 1.0)
               for g in ("g_q", "g_k", "g_v", "g_o"))


def _numpy_fallback(inputs):
    """Faithful numpy reimplementation (slow); used only for unexpected configs."""
    x = np.asarray(inputs["x"], np.float32)
    cos, sin = (np.asarray(inputs[k], np.float32) for k in ("freq_cos", "freq_sin"))
    causal = int(np.asarray(inputs["causal"]))

    def rms(t, g):
        n = t * (1.0 / np.sqrt(np.mean(t * t, -1, keepdims=True, dtype=np.float32) + EPS))
        return (g * n).astype(np.float32)

    def actq(t):
        scale = 127.0 / np.clip(np.max(np.abs(t), -1, keepdims=True), 1e-4, None)
        q = np.round(t * scale)
        return np.clip(q, -128, 127) / scale

    def ternq(w):
        s = np.mean(np.abs(w), dtype=np.float32)
        return np.round(np.tanh(w / (s + EPS))) * np.arctanh(s)

    def lin(t, w, g):
        return actq(rms(t, g)).astype(np.float32) @ ternq(np.asarray(w, np.float32)).T

    Bb, Ss, Dd = x.shape
    q = lin(x, inputs["w_q"], np.asarray(inputs["g_q"], np.float32)).reshape(Bb, Ss, H, HD)
    k = lin(x, inputs["w_k"], np.asarray(inputs["g_k"], np.float32)).reshape(Bb, Ss, KH, HD)
    v = lin(x, inputs["w_v"], np.asarray(inputs["g_v"], np.float32)).reshape(Bb, Ss, KH, HD)

    def rope(t):
        t2 = t.reshape(*t.shape[:-1], -1, 2)
        c = cos[None, :, None, :]
        s_ = sin[None, :, None, :]
        o0 = t2[..., 0] * c - t2[..., 1] * s_
        o1 = t2[..., 0] * s_ + t2[..., 1] * c
        return np.stack([o0, o1], -1).reshape(t.shape).astype(np.float32)

    q, k = rope(q), rope(k)
    scale = np.float32(HD ** 0.5)
    q = q.transpose(0, 2, 1, 3) / scale
    k = k.transpose(0, 2, 1, 3)
    v = v.transpose(0, 2, 1, 3)
    qg = q.reshape(Bb, 4, KH, Ss, HD).sum(1)
    sc = np.einsum("bhnd,bhsd->bhns", qg, k).astype(np.float32)
    if causal:
        mask = np.tril(np.ones((Ss, Ss), bool))
        sc = np.where(mask[None, None], sc, np.float32(np.finfo(np.float32).min))
    sc = sc / scale
    sc = sc - sc.max(-1, keepdims=True)
    p = np.exp(sc)
    p /= p.sum(-1, keepdims=True)
    out = np.einsum("bhns,bhsd->bnhd", p, v).reshape(Bb, Ss, KVD)
    return lin(out, inputs["w_o"], np.asarray(inputs["g_o"], np.float32))


def kernel(**inputs):
    x = np.asarray(inputs["x"])
    if x.shape != (B, S, D) or not _gains_trivial(inputs):
        return _numpy_fallback(inputs)
    causal = bool(int(np.asarray(inputs["causal"])))
    key = ("bitattn", causal)
    if key not in _cache:
        _cache[key] = build(causal)
    nc = _cache[key]
    in_maps = _prep_inputs(inputs)
    res = run_bass_kernel_spmd(nc, in_maps, core_ids=list(range(8)))
    y = np.empty((B, S, D), np.float32)
    for r in range(8):
        b, qq = r // 4, r % 4
        for tb in range(4):
            blk = 4 * tb + qq
            y[b, blk * 128:(blk + 1) * 128, :] = res.results[r]["y"][tb * 128:(tb + 1) * 128]
    return y


if __name__ == "__main__":
    data = np.load("/tmp/inputs.npz")
    inputs = {k: data[k] for k in data.files}
    out = kernel(**inputs)
    exp = np.load("/tmp/expected.npy")
    err = np.linalg.norm(out - exp) / np.linalg.norm(exp)
    print("Relative error:", err)


# revision 12
# speedup vs baseline: 1.6173x; 1.1534x over previous
"""BitAttention TRN2 kernel: 8-core SPMD (DP over batch x TP over kv-heads).

Self-contained: hardcodes shapes B=2, S=2048, D=2048, H=16, KH=4.
Core r: batch b = r//4, kv-head kh = r%4, stat/output stripe qq = r%4.

Math (forward-equivalent to the reference):
  - linear_bit = rms_norm -> per-row int8 act quant -> ternary weight quant
    -> matmul. Act-quant scale 127/max|xn| has rms self-cancel: the quantized
    ints are round(x*127/mx); rms enters only the per-token dequant scale.
  - Activations quantize straight to f16 with a +1536 offset (f16 ulp is 1 on
    [1024,2048) so the convert rounds half-to-even like jnp.round); the
    constant 1536 offset is removed inside the matmul by accumulating
    -1536*colsum(W) built from two exact f16 hi/lo matmuls.
  - Ternary weights quantize via round(clip(w*0.5/thr,-1,1)) (equivalent to
    round(tanh)), computed with the same +1536 f16 rounding trick.
  - The reference einsum sums the query-head group axis, so Q's 16 heads
    collapse to 4: group-sum the ternary w_q head blocks (ints in [-4,4]).
  - Scale folding: both 1/sqrt(HD) plus the wq/wk arctanh scales fold into
    the q rope tables (rope is linear); the wv scale cancels through the
    output rms-norm; the wo scale folds into the final dequant.
  - Attention runs transposed (S^T = K Q^T per key block) so softmaxed probs
    feed P^T directly into PV matmuls with no PE transposes; the softmax
    denominator comes from an extra all-ones column in the PV matmul.
    No max subtraction (logits are O(1) by construction).
  - Weight quantization work is sharded: batch-pair cores split w_q/w_k/w_v
    by row blocks, all 8 cores split w_o by columns; ternarized weights are
    exchanged with AllGather. Act-quant stats for x are sharded 4 ways across
    the batch group and AllGathered.
  - All tensor-engine transposes are done by DMA-transpose (f16/bf16).
"""
import numpy as np
from contextlib import ExitStack

import concourse.bass as bass
import concourse.bacc as bacc
import concourse.mybir as mybir
import concourse.tile as tile
from concourse.bass_utils import run_bass_kernel_spmd

B, S, D = 2, 2048, 2048
H, KH = 16, 4
HD = D // H          # 128
KVD = KH * HD        # 512
NB = S // 128        # 16 token blocks
SQ = S // 4          # 512 tokens per output stripe
EPS = 1e-8
MAGIC = float(1.5 * 2 ** 23)
M16 = 1536.0
ATANH05 = 0.5493061443340549      # arctanh(0.5)
NEG = -3.4e38
F32 = mybir.dt.float32
BF16 = mybir.dt.bfloat16
F16 = mybir.dt.float16
AX = mybir.AxisListType
OP = mybir.AluOpType
AF = mybir.ActivationFunctionType

_cache = {}


def _pt_off(kb, causal):
    if causal:
        return 2048 * kb - 64 * kb * (kb - 1)
    return 2048 * kb


def build(causal: bool, local_cc: bool = False):
    nc = bacc.Bacc()
    x_d = nc.dram_tensor("x", [S, D], F32, kind="ExternalInput")
    xs_d = nc.dram_tensor("xstat", [SQ, D], F32, kind="ExternalInput")
    wq_d = nc.dram_tensor("wq", [D // 2, KVD], F32, kind="ExternalInput")   # row-shard
    wk_d = nc.dram_tensor("wk", [D // 2, HD], F32, kind="ExternalInput")
    wv_d = nc.dram_tensor("wv", [D // 2, HD], F32, kind="ExternalInput")
    wo_d = nc.dram_tensor("wo", [KVD, D // 8], F32, kind="ExternalInput")   # col-shard
    cos_d = nc.dram_tensor("cos", [S, HD // 2], F32, kind="ExternalInput")
    sin_d = nc.dram_tensor("sin", [S, HD // 2], F32, kind="ExternalInput")
    qsel_d = nc.dram_tensor("qsel", [128, 2], F32, kind="ExternalInput")
    y_d = nc.dram_tensor("y", [SQ, D], F32, kind="ExternalOutput")
    st_in = nc.dram_tensor("st_in", [1, 4], F32)
    st_out = nc.dram_tensor("st_out", [1, 4], F32, addr_space="Shared")
    sc_in = nc.dram_tensor("sc_in", [4, 128, 2], F32)
    sc_out = nc.dram_tensor("sc_out", [4, 4, 128, 2], F32, addr_space="Shared")
    wg_in = nc.dram_tensor("wg_in", [8, 128, 3 * HD], F16)
    wg_out = nc.dram_tensor("wg_out", [2, 8, 128, 3 * HD], F16, addr_space="Shared")
    wob_in = nc.dram_tensor("wob_in", [4, 128, 256], BF16)
    wob_out = nc.dram_tensor("wob_out", [8, 4, 128, 256], BF16, addr_space="Shared")
    cc_in = nc.dram_tensor("cc_in", [4, 8, 128, HD], F32)
    cc_out = nc.dram_tensor("cc_out", [4, 8, 128, HD], F32, addr_space="Shared")

    PTW = _pt_off(NB, causal)

    with tile.TileContext(nc) as tc, ExitStack() as ctx:
        cpool = ctx.enter_context(tc.tile_pool(name="const", bufs=1))
        sm = ctx.enter_context(tc.tile_pool(name="sm", bufs=1))
        wres = ctx.enter_context(tc.tile_pool(name="wres", bufs=1))

        # ---------- constants ----------
        cmT = cpool.tile([128, 128], F32, tag="cmT")
        if causal:
            nc.gpsimd.memset(cmT[:], 0.0)
            nc.gpsimd.affine_select(out=cmT[:], in_=cmT[:], compare_op=OP.is_ge,
                                    fill=NEG, base=0, pattern=[[1, 128]],
                                    channel_multiplier=-1)
        ones_f16 = cpool.tile([128, 1], F16, tag="o16")
        nc.any.memset(ones_f16[:], 1.0)
        ones_bf = cpool.tile([128, 1], BF16, tag="obf")
        nc.any.memset(ones_bf[:], 1.0)
        ones_c = cpool.tile([128, 1], F32, tag="onc")
        nc.any.memset(ones_c[:], 1.0)
        ones_r = cpool.tile([1, 128], F32, tag="onr")
        nc.any.memset(ones_r[:], 1.0)
        n192 = cpool.tile([128, 128], F16, tag="n192")
        nc.any.memset(n192[:], -192.0)
        n12 = cpool.tile([128, 128], F16, tag="n12")
        nc.any.memset(n12[:], -12.0)
        inv_n = cpool.tile([128, 4], F32, tag="invn")
        for j, numel in enumerate([D * D, KVD * D, KVD * D, D * KVD]):
            nc.any.memset(inv_n[:, j:j + 1], 1.0 / numel)
        sqscr = cpool.tile([128, D], BF16, tag="sqscr")
        cos_kb = cpool.tile([128, NB, HD // 2], BF16, tag="coskb")
        sin_kb = cpool.tile([128, NB, HD // 2], BF16, tag="sinkb")
        cos_qb = cpool.tile([128, NB, HD // 2], BF16, tag="cosqb")
        sin_qb = cpool.tile([128, NB, HD // 2], BF16, tag="sinqb")

        # persistent smalls
        pr = sm.tile([128, 4], F32, tag="pr")
        st_sb = sm.tile([1, 4], F32, tag="st_sb")
        st2_sb = sm.tile([1, 4], F32, tag="st2_sb")
        totals = sm.tile([128, 4], F32, tag="totals")
        s4 = sm.tile([128, 4], F32, tag="s4")
        hi4 = sm.tile([128, 4], F32, tag="hi4")
        a4 = sm.tile([128, 4], F32, tag="a4")
        aqk = sm.tile([128, 1], F32, tag="aqk")
        mxs = sm.tile([128, 4], F32, tag="mxs")
        ssqs = sm.tile([128, 4], F32, tag="ssqs")
        sx_sb = sm.tile([128, 4, 2], F32, tag="sx_sb")
        sd_all = sm.tile([128, 16, 2], F32, tag="sd_all")
        csr = sm.tile([1, 3 * HD], F32, tag="csr")
        csbc = sm.tile([128, 3 * HD], F32, tag="csbc")
        hq = sm.tile([128, 3 * HD], F16, tag="hq")
        lq = sm.tile([128, 3 * HD], F16, tag="lq")

        # persistent quantized weights
        wqkv = wres.tile([128, NB, 3 * HD], F16, tag="wqkv", name="wqkv")
        wo_q = wres.tile([128, 4, D], BF16, tag="wo_q", name="wo_q")

        # cos/sin staging (f32, freed after table builds)
        with tc.tile_pool(name="cstage", bufs=1) as cstage:
            cosf = cstage.tile([128, NB, HD // 2], F32, tag="cosf")
            sinf = cstage.tile([128, NB, HD // 2], F32, tag="sinf")
            nc.sync.dma_start(cosf[:], cos_d.ap().rearrange("(i p) f -> p i f", p=128))
            nc.sync.dma_start(sinf[:], sin_d.ap().rearrange("(i p) f -> p i f", p=128))
            nc.vector.tensor_copy(cos_kb[:], cosf[:])
            nc.vector.tensor_copy(sin_kb[:], sinf[:])

            # ---------- weight phase (sharded) ----------
            with tc.tile_pool(name="wph", bufs=1) as wph:
                wq_sb = wph.tile([128, 8, KVD], F32, tag="wq_sb")
                wk_sb = wph.tile([128, 8, HD], F32, tag="wk_sb")
                wv_sb = wph.tile([128, 8, HD], F32, tag="wv_sb")
                wo_sb = wph.tile([128, 4, 256], F32, tag="wo_sb")
                nc.sync.dma_start(wq_sb[:], wq_d.ap().rearrange("(j p) c -> p j c", p=128))
                nc.sync.dma_start(wk_sb[:], wk_d.ap().rearrange("(j p) c -> p j c", p=128))
                nc.sync.dma_start(wv_sb[:], wv_d.ap().rearrange("(j p) c -> p j c", p=128))
                nc.sync.dma_start(wo_sb[:], wo_d.ap().rearrange("(c p) d -> p c d", p=128))

                # pass1: |w| row sums (each element once across the fleet)
                nc.vector.tensor_reduce(pr[:, 0:1], wq_sb[:].rearrange("p a b -> p (a b)"),
                                        axis=AX.X, op=OP.add, apply_absolute_value=True)
                nc.vector.tensor_reduce(pr[:, 1:2], wk_sb[:].rearrange("p a b -> p (a b)"),
                                        axis=AX.X, op=OP.add, apply_absolute_value=True)
                nc.vector.tensor_reduce(pr[:, 2:3], wv_sb[:].rearrange("p a b -> p (a b)"),
                                        axis=AX.X, op=OP.add, apply_absolute_value=True)
                nc.vector.tensor_reduce(pr[:, 3:4], wo_sb[:].rearrange("p a b -> p (a b)"),
                                        axis=AX.X, op=OP.add, apply_absolute_value=True)

                with tc.tile_pool(name="psst", bufs=2, space="PSUM") as psst:
                    pcol = psst.tile([1, 4], F32, tag="st")
                    nc.tensor.matmul(pcol[:], ones_c[:], pr[:], start=True, stop=True)
                    nc.vector.tensor_copy(st_sb[:], pcol[:])
                    nc.scalar.dma_start(st_in[:], st_sb[:])
                    if local_cc:
                        nc.scalar.dma_start(st_out.ap(), st_in.ap())
                    else:
                        nc.gpsimd.collective_compute(
                            "AllReduce", OP.add, replica_groups=[list(range(8))],
                            ins=[st_in.ap().opt()], outs=[st_out.ap().opt()])
                    nc.scalar.dma_start(st2_sb[:], st_out[:])
                    bc = psst.tile([128, 4], F32, tag="st")
                    nc.tensor.matmul(bc[:], ones_r[:], st2_sb[:], start=True, stop=True)
                    nc.vector.tensor_copy(totals[:], bc[:])

                # s, 1/(2 thr), a  (all [128,4] replicated)
                nc.vector.tensor_tensor(s4[:], totals[:], inv_n[:], op=OP.mult)
                thr2 = sm.tile([128, 4], F32, tag="thr2")
                nc.vector.tensor_scalar(thr2[:], s4[:], EPS, 2.0 * ATANH05,
                                        op0=OP.add, op1=OP.mult)
                nc.vector.reciprocal(hi4[:], thr2[:])
                ntp = sm.tile([128, 4], F32, tag="ntp")
                nc.vector.tensor_tensor(ntp[:], thr2[:], hi4[:], op=OP.mult)
                nc.vector.tensor_scalar(ntp[:], ntp[:], -1.0, 2.0, op0=OP.mult, op1=OP.add)
                nc.vector.tensor_tensor(hi4[:], hi4[:], ntp[:], op=OP.mult)
                num = sm.tile([128, 4], F32, tag="num")
                den = sm.tile([128, 4], F32, tag="den")
                rat = sm.tile([128, 4], F32, tag="rat")
                nc.vector.tensor_scalar(num[:], s4[:], 1.0, None, op0=OP.add)
                nc.vector.tensor_scalar(den[:], s4[:], -1.0, 1.0, op0=OP.mult, op1=OP.add)
                nc.vector.reciprocal(rat[:], den[:])
                nc.vector.tensor_tensor(ntp[:], den[:], rat[:], op=OP.mult)
                nc.vector.tensor_scalar(ntp[:], ntp[:], -1.0, 2.0, op0=OP.mult, op1=OP.add)
                nc.vector.tensor_tensor(rat[:], rat[:], ntp[:], op=OP.mult)
                nc.vector.tensor_tensor(rat[:], rat[:], num[:], op=OP.mult)
                lnr = sm.tile([128, 4], F32, tag="lnr")
                nc.scalar.activation(lnr[:], rat[:], AF.Ln)
                nc.vector.tensor_scalar(a4[:], lnr[:], 0.5, None, op0=OP.mult)
                nc.vector.tensor_tensor(aqk[:], a4[:, 0:1], a4[:, 1:2], op=OP.mult)
                nc.vector.tensor_scalar(aqk[:], aqk[:], 1.0 / HD, None, op0=OP.mult)
                # scaled q rope tables (fold a_q*a_k/HD into q's rotation)
                nc.vector.tensor_scalar(cos_qb[:], cosf[:], aqk[:], None, op0=OP.mult)
                nc.vector.tensor_scalar(sin_qb[:], sinf[:], aqk[:], None, op0=OP.mult)

                # ternary quantize shards.  passA (pool): u = min(w*hi, 1)
                # passB (DVE): f16(max(u,-1) + 1536); passC: sub 1536
                wsh = wph.tile([128, 8, 3 * HD], F16, tag="wsh")
                wosh = wph.tile([128, 4, 256], BF16, tag="wosh")
                tq = wph.tile([128, 8, KVD], F16, tag="tq")
                tk = wph.tile([128, 8, HD], F16, tag="tk")
                tg1 = wph.tile([128, 8, HD], F16, tag="tg1")
                tg2 = wph.tile([128, 8, HD], F16, tag="tg2")

                def ternA(t, col):
                    nc.gpsimd.tensor_scalar(t[:], t[:], hi4[:, col:col + 1], 1.0,
                                            op0=OP.mult, op1=OP.min)

                # wq's passA on DVE: it heads the critical path to the gather
                nc.vector.tensor_scalar(wq_sb[:], wq_sb[:], hi4[:, 0:1], 1.0,
                                        op0=OP.mult, op1=OP.min)
                nc.vector.tensor_scalar(tq[:], wq_sb[:], -1.0, M16, op0=OP.max, op1=OP.add)
                nc.vector.tensor_scalar(tq[:], tq[:], M16, None, op0=OP.subtract)
                # group-sum 4 head blocks (ints in [-4,4], exact f16)
                nc.vector.tensor_tensor(tg1[:], tq[:, :, 0:HD], tq[:, :, HD:2 * HD], op=OP.add)
                nc.vector.tensor_tensor(tg2[:], tq[:, :, 2 * HD:3 * HD], tq[:, :, 3 * HD:4 * HD], op=OP.add)
                nc.vector.tensor_tensor(wsh[:, :, 0:HD], tg1[:], tg2[:], op=OP.add)
                ternA(wk_sb, 1)
                nc.vector.tensor_scalar(tk[:], wk_sb[:], -1.0, M16, op0=OP.max, op1=OP.add)
                nc.vector.tensor_scalar(wsh[:, :, HD:2 * HD], tk[:], M16, None, op0=OP.subtract)
                ternA(wv_sb, 2)
                nc.vector.tensor_scalar(tk[:], wv_sb[:], -1.0, M16, op0=OP.max, op1=OP.add)
                nc.vector.tensor_scalar(wsh[:, :, 2 * HD:3 * HD], tk[:], M16, None, op0=OP.subtract)
                ternA(wo_sb, 3)
                to16 = wph.tile([128, 4, 256], F16, tag="to16")
                nc.vector.tensor_scalar(to16[:], wo_sb[:], -1.0, M16, op0=OP.max, op1=OP.add)
                nc.vector.tensor_scalar(wosh[:], to16[:], M16, None, op0=OP.subtract)

                # exchange ternary shards (ACT-issued DMAs to keep SP free)
                nc.scalar.dma_start(wg_in.ap().rearrange("j p c -> p j c"), wsh[:])
                nc.scalar.dma_start(wob_in.ap().rearrange("c p d -> p c d"), wosh[:])
                if local_cc:
                    nc.scalar.dma_start(wg_out.ap()[0], wg_in.ap())
                    nc.scalar.dma_start(wob_out.ap()[0], wob_in.ap())
                else:
                    nc.gpsimd.collective_compute(
                        "AllGather", OP.bypass,
                        replica_groups=[[0, 4], [1, 5], [2, 6], [3, 7]],
                        ins=[wg_in.ap().opt()], outs=[wg_out.ap().opt()])
                    nc.gpsimd.collective_compute(
                        "AllGather", OP.bypass, replica_groups=[list(range(8))],
                        ins=[wob_in.ap().opt()], outs=[wob_out.ap().opt()])
                for s in range(2):
                    nc.scalar.dma_start(
                        wqkv[:].rearrange("p (j s) c -> p j s c", s=2)[:, :, s, :],
                        wg_out.ap()[s].rearrange("j p c -> p j c"))
                for s in range(8):
                    nc.scalar.dma_start(
                        wo_q[:].rearrange("p c (s d) -> p c s d", s=8)[:, :, s, :],
                        wob_out.ap()[s].rearrange("c p d -> p c d"))

        # ---------- x stats (sharded 4-way over the batch group) ----------
        with tc.tile_pool(name="xph", bufs=1) as xph:
            for i in range(4):
                xs = xph.tile([128, D], F32, tag="xb", bufs=3)
                nc.sync.dma_start(xs[:], xs_d[i * 128:(i + 1) * 128, :])
                nc.vector.tensor_reduce(mxs[:, i:i + 1], xs[:], axis=AX.X,
                                        op=OP.max, apply_absolute_value=True)
                nc.scalar.activation(sqscr[:], xs[:], AF.Square,
                                     accum_out=ssqs[:, i:i + 1])
            mean4 = sm.tile([128, 4], F32, tag="mean4")
            nc.vector.tensor_scalar(mean4[:], ssqs[:], 1.0 / D, EPS, op0=OP.mult, op1=OP.add)
            sd4 = sm.tile([128, 4], F32, tag="sd4")
            nc.scalar.activation(sd4[:], mean4[:], AF.Sqrt)
            r4 = sm.tile([128, 4], F32, tag="r4")
            nc.vector.reciprocal(r4[:], sd4[:])
            nt4 = sm.tile([128, 4], F32, tag="nt4")
            nc.vector.tensor_tensor(nt4[:], sd4[:], r4[:], op=OP.mult)
            nc.vector.tensor_scalar(nt4[:], nt4[:], -1.0, 2.0, op0=OP.mult, op1=OP.add)
            nc.vector.tensor_tensor(r4[:], r4[:], nt4[:], op=OP.mult)
            m127 = sm.tile([128, 4], F32, tag="m127")
            nc.vector.tensor_scalar(m127[:], mxs[:], 1.0 / 127.0, None, op0=OP.mult)
            smul4 = sm.tile([128, 4], F32, tag="smul4")
            nc.vector.reciprocal(smul4[:], m127[:])
            nc.vector.tensor_tensor(nt4[:], m127[:], smul4[:], op=OP.mult)
            nc.vector.tensor_scalar(nt4[:], nt4[:], -1.0, 2.0, op0=OP.mult, op1=OP.add)
            nc.vector.tensor_tensor(smul4[:], smul4[:], nt4[:], op=OP.mult)
            deq4 = sm.tile([128, 4], F32, tag="deq4")
            nc.vector.tensor_tensor(deq4[:], mxs[:], r4[:], op=OP.mult)
            nc.vector.tensor_scalar(deq4[:], deq4[:], 1.0 / 127.0, None, op0=OP.mult)
            nc.vector.tensor_copy(sx_sb[:, :, 0], smul4[:])
            nc.vector.tensor_copy(sx_sb[:, :, 1], deq4[:])
            nc.sync.dma_start(sc_in.ap().rearrange("i p c -> p i c"), sx_sb[:])
            if local_cc:
                nc.sync.dma_start(sc_out.ap()[0], sc_in.ap())
            else:
                nc.gpsimd.collective_compute(
                    "AllGather", OP.bypass,
                    replica_groups=[[0, 1, 2, 3], [4, 5, 6, 7]],
                    ins=[sc_in.ap().opt()], outs=[sc_out.ap().opt()])
            nc.sync.dma_start(sd_all[:], sc_out.ap().rearrange("s i p c -> p (s i) c"))

            # ---------- x quantize + transpose (4-block chunks) ----------
            with tc.tile_pool(name="xqTp", bufs=1) as xqTp:
                xqT = xqTp.tile([128, NB * NB, 128], F16, tag="xqT")
                for ci in range(4):
                    qch = xph.tile([128, 4, D], F16, tag="qch", bufs=2)
                    for ib in range(4):
                        i = 4 * ci + ib
                        xb = xph.tile([128, D], F32, tag="xb", bufs=3)
                        nc.sync.dma_start(xb[:], x_d[i * 128:(i + 1) * 128, :])
                        hw = D // 2
                        nc.vector.tensor_scalar(qch[:, ib, 0:hw], xb[:, 0:hw],
                                                sd_all[:, i, 0:1], M16,
                                                op0=OP.mult, op1=OP.add)
                        nc.scalar.activation(qch[:, ib, hw:D], xb[:, hw:D], AF.Copy,
                                             bias=M16, scale=sd_all[:, i, 0:1])
                    nc.sync.dma_start_transpose(
                        xqT[:, 64 * ci:64 * (ci + 1), :],
                        qch[:].rearrange("p a b -> p (a b)"))

                # csum of wqkv (for the -1536 offset correction), f16 hi/lo split
                qkv_all = sm.tile([128, NB, 3 * HD], BF16, tag="qkv_all", name="qkv_all")
                with tc.tile_pool(name="psc", bufs=2, space="PSUM") as psc:
                    csp = psc.tile([1, 3 * HD], F32, tag="cs")
                    for j in range(NB):
                        nc.tensor.matmul(csp[:], ones_f16[:], wqkv[:, j, :],
                                         start=(j == 0), stop=(j == NB - 1))
                    nc.vector.tensor_copy(csr[:], csp[:])
                    bcp = psc.tile([128, 3 * HD], F32, tag="cs")
                    nc.tensor.matmul(bcp[:], ones_r[:], csr[:], start=True, stop=True)
                    nc.vector.tensor_copy(csbc[:], bcp[:])
                    tcs = sm.tile([128, 3 * HD], F32, tag="tcs")
                    nc.vector.tensor_scalar(tcs[:], csbc[:], 1.0 / 16.0, MAGIC,
                                            op0=OP.mult, op1=OP.add)
                    nc.vector.tensor_scalar(tcs[:], tcs[:], MAGIC, None, op0=OP.subtract)
                    nc.vector.tensor_copy(hq[:], tcs[:])
                    nc.vector.scalar_tensor_tensor(lq[:], tcs[:], -16.0, csbc[:],
                                                   op0=OP.mult, op1=OP.add)

                    # ---------- QKV projections ----------
                    for i in range(NB):
                        pq = psc.tile([128, 3 * HD], F32, tag="mm")
                        for j in range(NB):
                            nc.tensor.matmul(pq[:], xqT[:, 16 * i + j, :],
                                             wqkv[:, j, :], start=(j == 0), stop=False)
                        nc.tensor.matmul(pq[:], n192[:], hq[:], start=False, stop=False,
                                         skip_group_check=True)
                        nc.tensor.matmul(pq[:], n12[:], lq[:], start=False, stop=True,
                                         skip_group_check=True)
                        nc.scalar.activation(qkv_all[:, i, :], pq[:], AF.Copy,
                                             scale=sd_all[:, i, 1:2])

        # ---------- rope (4-block chunks, bf16) + transposes ----------
        atl = ctx.enter_context(tc.tile_pool(name="atl", bufs=1))
        qT = atl.tile([128, NB, 128], BF16, tag="qT", name="qT")
        kT = atl.tile([128, NB, 128], BF16, tag="kT", name="kT")
        with tc.tile_pool(name="rp", bufs=1) as rp:
            hh = HD // 2
            qr = rp.tile([128, NB, HD], BF16, tag="qr")
            kr = rp.tile([128, NB, HD], BF16, tag="kr")
            for ci in range(4):
                cs = slice(4 * ci, 4 * ci + 4)
                for src0, cosb, sinb, dst in ((0, cos_qb, sin_qb, qr),
                                              (HD, cos_kb, sin_kb, kr)):
                    ev = qkv_all[:, cs, src0:src0 + hh]
                    od = qkv_all[:, cs, src0 + hh:src0 + HD]
                    t1 = rp.tile([128, 4, hh], BF16, tag="t1", bufs=2)
                    t2 = rp.tile([128, 4, hh], BF16, tag="t2", bufs=2)
                    nc.vector.tensor_tensor(t1[:], ev, cosb[:, cs, :], op=OP.mult)
                    nc.vector.tensor_tensor(t2[:], od, sinb[:, cs, :], op=OP.mult)
                    nc.vector.tensor_tensor(dst[:, cs, 0:hh], t1[:], t2[:], op=OP.subtract)
                    t3 = rp.tile([128, 4, hh], BF16, tag="t1", bufs=2)
                    t4 = rp.tile([128, 4, hh], BF16, tag="t2", bufs=2)
                    nc.vector.tensor_tensor(t3[:], ev, sinb[:, cs, :], op=OP.mult)
                    nc.vector.tensor_tensor(t4[:], od, cosb[:, cs, :], op=OP.mult)
                    nc.vector.tensor_tensor(dst[:, cs, hh:HD], t3[:], t4[:], op=OP.add)
                nc.sync.dma_start_transpose(
                    qT[:, cs, :], qr[:, cs, :].rearrange("p a b -> p (a b)"))
                nc.sync.dma_start_transpose(
                    kT[:, cs, :], kr[:, cs, :].rearrange("p a b -> p (a b)"))

        # ---------- attention + output projection ----------
        qTf = qT[:].rearrange("p a b -> p (a b)")
        with tc.tile_pool(name="attn", bufs=1) as attn, \
             tc.tile_pool(name="pss", bufs=3, space="PSUM") as pss, \
             tc.tile_pool(name="psv", bufs=2, space="PSUM") as psv, \
             tc.tile_pool(name="psy", bufs=2, space="PSUM") as psy:
            PT = attn.tile([128, PTW], BF16, tag="PT")
            obuf = attn.tile([128, 4, HD], F32, tag="obuf", bufs=2)

            def scores(kb):
                qlo = 128 * kb if causal else 0
                c0 = qlo
                first = True
                while c0 < S:
                    cw = min(512, S - c0)
                    sp = pss.tile([128, 512], F32, tag="sc")
                    nc.tensor.matmul(sp[:, 0:cw], kT[:, kb, :], qTf[:, c0:c0 + cw],
                                     start=True, stop=True)
                    if causal and first:
                        nc.vector.tensor_tensor(sp[:, 0:128], sp[:, 0:128], cmT[:],
                                                op=OP.add)
                    nc.scalar.activation(PT[:, _pt_off(kb, causal) + c0 - qlo:
                                            _pt_off(kb, causal) + c0 - qlo + cw],
                                         sp[:, 0:cw], AF.Exp)
                    first = False
                    c0 += cw

            def pv(qb):
                po = psv.tile([128, 132], F32, tag="po")
                nkb = qb + 1 if causal else NB
                for k2 in range(nkb):
                    qoff = (qb - k2) * 128 if causal else qb * 128
                    lhs = PT[:, _pt_off(k2, causal) + qoff:
                             _pt_off(k2, causal) + qoff + 128]
                    nc.tensor.matmul(po[:, 0:HD], lhs, qkv_all[:, k2, 2 * HD:3 * HD],
                                     start=(k2 == 0), stop=(k2 == nkb - 1),
                                     skip_group_check=True)
                    nc.tensor.matmul(po[:, HD:HD + 1], lhs, ones_bf[:],
                                     start=(k2 == 0), stop=(k2 == nkb - 1),
                                     skip_group_check=True)
                rz = attn.tile([128, 1], F32, tag="rz", bufs=2)
                nz = attn.tile([128, 1], F32, tag="nz", bufs=2)
                nc.vector.reciprocal(rz[:], po[:, HD:HD + 1])
                nc.vector.tensor_tensor(nz[:], po[:, HD:HD + 1], rz[:], op=OP.mult)
                nc.vector.tensor_scalar(nz[:], nz[:], -1.0, 2.0, op0=OP.mult, op1=OP.add)
                nc.vector.tensor_tensor(rz[:], rz[:], nz[:], op=OP.mult)
                nc.scalar.activation(obuf[:, qb % 4, :], po[:, 0:HD], AF.Copy,
                                     scale=rz[:])

            xo8s = [None] * 4

            def cc_ex(tb):
                nc.sync.dma_start(cc_in.ap()[tb, 0:4].rearrange("s p d -> p s d"),
                                  obuf[:])
                nc.sync.dma_start(cc_in.ap()[tb, 4:8].rearrange("s p d -> p s d"),
                                  obuf[:])
                if local_cc:
                    nc.sync.dma_start(cc_out.ap()[tb], cc_in.ap()[tb])
                else:
                    nc.gpsimd.collective_compute(
                        "AllToAll", OP.bypass, replica_groups=[list(range(8))],
                        ins=[cc_in.ap()[tb].opt()], outs=[cc_out.ap()[tb].opt()])
                xo8 = attn.tile([128, 8, HD], F32, tag="xo8", bufs=2)
                nc.sync.dma_start(xo8[:], cc_out.ap()[tb].rearrange("s p d -> p s d"))
                xo8s[tb] = xo8

            def oproj(tb):
                xo8 = xo8s[tb]
                xsel = attn.tile([128, KVD], F32, tag="xsel", bufs=2)
                nc.gpsimd.tensor_scalar(xsel[:], xo8[:, 0:4, :].rearrange("p a b -> p (a b)"),
                                        qsel[:, 0:1], None, op0=OP.mult)
                xo = attn.tile([128, KVD], F32, tag="xo", bufs=2)
                nc.gpsimd.scalar_tensor_tensor(xo[:], xo8[:, 4:8, :].rearrange("p a b -> p (a b)"),
                                               qsel[:, 1:2], xsel[:],
                                               op0=OP.mult, op1=OP.add)
                mx2 = attn.tile([128, 1], F32, tag="mx2", bufs=2)
                nc.vector.tensor_reduce(mx2[:], xo[:], axis=AX.X, op=OP.max,
                                        apply_absolute_value=True)
                ssq2 = attn.tile([128, 1], F32, tag="ssq2", bufs=2)
                nc.scalar.activation(sqscr[:, 0:KVD], xo[:], AF.Square, accum_out=ssq2[:])
                mean2 = attn.tile([128, 1], F32, tag="mean2", bufs=2)
                nc.vector.tensor_scalar(mean2[:], ssq2[:], 1.0 / KVD, EPS,
                                        op0=OP.mult, op1=OP.add)
                sd2 = attn.tile([128, 1], F32, tag="sd2", bufs=2)
                nc.scalar.activation(sd2[:], mean2[:], AF.Sqrt)
                r2 = attn.tile([128, 1], F32, tag="r2", bufs=2)
                nt2 = attn.tile([128, 1], F32, tag="nt2", bufs=2)
                nc.vector.reciprocal(r2[:], sd2[:])
                nc.vector.tensor_tensor(nt2[:], sd2[:], r2[:], op=OP.mult)
                nc.vector.tensor_scalar(nt2[:], nt2[:], -1.0, 2.0, op0=OP.mult, op1=OP.add)
                nc.vector.tensor_tensor(r2[:], r2[:], nt2[:], op=OP.mult)
                m2 = attn.tile([128, 1], F32, tag="m2", bufs=2)
                nc.vector.tensor_scalar(m2[:], mx2[:], 1.0 / 127.0, None, op0=OP.mult)
                sl2 = attn.tile([128, 1], F32, tag="sl2", bufs=2)
                nc.vector.reciprocal(sl2[:], m2[:])
                nc.vector.tensor_tensor(nt2[:], m2[:], sl2[:], op=OP.mult)
                nc.vector.tensor_scalar(nt2[:], nt2[:], -1.0, 2.0, op0=OP.mult, op1=OP.add)
                nc.vector.tensor_tensor(sl2[:], sl2[:], nt2[:], op=OP.mult)
                dqy = attn.tile([128, 1], F32, tag="dqy", bufs=2)
                nc.vector.tensor_tensor(dqy[:], mx2[:], r2[:], op=OP.mult)
                nc.vector.tensor_scalar(dqy[:], dqy[:], 1.0 / 127.0, None, op0=OP.mult)
                nc.vector.tensor_tensor(dqy[:], dqy[:], a4[:, 3:4], op=OP.mult)
                nc.vector.tensor_scalar(xo[:], xo[:], sl2[:], MAGIC,
                                        op0=OP.mult, op1=OP.add)
                qo = attn.tile([128, KVD], BF16, tag="qo", bufs=2)
                nc.scalar.activation(qo[:], xo[:], AF.Copy, bias=-MAGIC)
                xoT = attn.tile([128, 4, 128], BF16, tag="xoT", bufs=2)
                nc.sync.dma_start_transpose(xoT[:], qo[:])
                y_sb = attn.tile([128, D], F32, tag="ysb", bufs=2)
                for oc in range(4):
                    py = psy.tile([128, 512], F32, tag="my")
                    for jc in range(4):
                        nc.tensor.matmul(py[:], xoT[:, jc, :],
                                         wo_q[:, jc, oc * 512:(oc + 1) * 512],
                                         start=(jc == 0), stop=(jc == 3))
                    if oc % 2 == 0:
                        nc.scalar.activation(y_sb[:, oc * 512:(oc + 1) * 512], py[:],
                                             AF.Copy, scale=dqy[:])
                    else:
                        nc.vector.tensor_scalar(y_sb[:, oc * 512:(oc + 1) * 512],
                                                py[:], dqy[:], None, op0=OP.mult)
                nc.sync.dma_start(y_d[tb * 128:(tb + 1) * 128, :], y_sb[:])

            qsel = cpool.tile([128, 2], F32, tag="qsel")
            nc.sync.dma_start(qsel[:], qsel_d[:])

            def post_pv(qb):
                # at quarter completion: launch the exchange; run the PREVIOUS
                # quarter's output projection (its data has long arrived)
                if qb % 4 == 3:
                    tb = qb // 4
                    cc_ex(tb)
                    if tb >= 1:
                        oproj(tb - 1)

            if causal:
                scores(0)
                for kb in range(1, NB):
                    scores(kb)
                    pv(kb - 1)
                    post_pv(kb - 1)
                pv(NB - 1)
                post_pv(NB - 1)
            else:
                for kb in range(NB):
                    scores(kb)
                for qb in range(NB):
                    pv(qb)
                    post_pv(qb)
            oproj(3)
    nc.compile()
    return nc


def _rope_perm():
    p = np.empty(HD, np.int64)
    p[:HD // 2] = np.arange(0, HD, 2)
    p[HD // 2:] = np.arange(1, HD, 2)
    return p


def qsel_host(b):
    q = np.zeros((128, 2), np.float32)
    q[:, b] = 1.0
    return q


def _prep_inputs(inputs):
    x = np.ascontiguousarray(np.asarray(inputs["x"], np.float32))
    w_q = np.asarray(inputs["w_q"], np.float32)
    w_k = np.asarray(inputs["w_k"], np.float32)
    w_v = np.asarray(inputs["w_v"], np.float32)
    w_o = np.asarray(inputs["w_o"], np.float32)
    cos = np.ascontiguousarray(np.asarray(inputs["freq_cos"], np.float32))
    sin = np.ascontiguousarray(np.asarray(inputs["freq_sin"], np.float32))
    perm = _rope_perm()
    woT = np.ascontiguousarray(w_o.T)                      # [KVD, D]
    in_maps = []
    jrows = np.arange(D) // 128 % 2
    for r in range(8):
        b, kh = r // 4, r % 4
        heads = [g * KH + kh for g in range(4)]
        wq_sel = w_q.reshape(H, HD, D)[heads][:, perm, :]  # [4,128,D]
        wqT = np.ascontiguousarray(wq_sel.reshape(4 * HD, D).T)   # [D, 512]
        wkT = np.ascontiguousarray(w_k[kh * HD:(kh + 1) * HD][perm].T)  # [D,128]
        wvT = np.ascontiguousarray(w_v[kh * HD:(kh + 1) * HD].T)        # [D,128]
        sel = jrows == b
        in_maps.append({
            "x": x[b],
            "xstat": np.ascontiguousarray(x[b][kh * SQ:(kh + 1) * SQ]),
            "wq": np.ascontiguousarray(wqT[sel]),
            "wk": np.ascontiguousarray(wkT[sel]),
            "wv": np.ascontiguousarray(wvT[sel]),
            "wo": np.ascontiguousarray(woT[:, r * 256:(r + 1) * 256]),
            "cos": cos, "sin": sin,
            "qsel": qsel_host(b),
        })
    return in_maps


def _gains_trivial(inputs):
    return all(np.all(np.asarray(inputs[g]) == 1.0)
               for g in ("g_q", "g_k", "g_v", "g_o"))


def _numpy_fallback(inputs):
    """Faithful numpy reimplementation (slow); used only for unexpected configs."""
    x = np.asarray(inputs["x"], np.float32)
    cos, sin = (np.asarray(inputs[k], np.float32) for k in ("freq_cos", "freq_sin"))
    causal = int(np.asarray(inputs["causal"]))

    def rms(t, g):
        n = t * (1.0 / np.sqrt(np.mean(t * t, -1, keepdims=True, dtype=np.float32) + EPS))
        return (g * n).astype(np.float32)

    def actq(t):
        scale = 127.0 / np.clip(np.max(np.abs(t), -1, keepdims=True), 1e-4, None)
        q = np.round(t * scale)
        return np.clip(q, -128, 127) / scale

    def ternq(w):
        s = np.mean(np.abs(w), dtype=np.float32)
        return np.round(np.tanh(w / (s + EPS))) * np.arctanh(s)

    def lin(t, w, g):
        return actq(rms(t, g)).astype(np.float32) @ ternq(np.asarray(w, np.float32)).T

    Bb, Ss, Dd = x.shape
    q = lin(x, inputs["w_q"], np.asarray(inputs["g_q"], np.float32)).reshape(Bb, Ss, H, HD)
    k = lin(x, inputs["w_k"], np.asarray(inputs["g_k"], np.float32)).reshape(Bb, Ss, KH, HD)
    v = lin(x, inputs["w_v"], np.asarray(inputs["g_v"], np.float32)).reshape(Bb, Ss, KH, HD)

    def rope(t):
        t2 = t.reshape(*t.shape[:-1], -1, 2)
        c = cos[None, :, None, :]
        s_ = sin[None, :, None, :]
        o0 = t2[..., 0] * c - t2[..., 1] * s_
        o1 = t2[..., 0] * s_ + t2[..., 1] * c
        return np.stack([o0, o1], -1).reshape(t.shape).astype(np.float32)

    q, k = rope(q), rope(k)
    scale = np.float32(HD ** 0.5)
    q = q.transpose(0, 2, 1, 3) / scale
    k = k.transpose(0, 2, 1, 3)
    v = v.transpose(0, 2, 1, 3)
    qg = q.reshape(Bb, 4, KH, Ss, HD).sum(1)
    sc = np.einsum("bhnd,bhsd->bhns", qg, k).astype(np.float32)
    if causal:
        mask = np.tril(np.ones((Ss, Ss), bool))
        sc = np.where(mask[None, None], sc, np.float32(np.finfo(np.float32).min))
    sc = sc / scale
    sc = sc - sc.max(-1, keepdims=True)
    p = np.exp(sc)
    p /= p.sum(-1, keepdims=True)
    out = np.einsum("bhns,bhsd->bnhd", p, v).reshape(Bb, Ss, KVD)
    return lin(out, inputs["w_o"], np.asarray(inputs["g_o"], np.float32))


def kernel(**inputs):
    x = np.asarray(inputs["x"])
    if x.shape != (B, S, D) or not _gains_trivial(inputs):
        return _numpy_fallback(inputs)
    causal = bool(int(np.asarray(inputs["causal"])))
    key = ("bitattn", causal)
    if key not in _cache:
        _cache[key] = build(causal)
    nc = _cache[key]
    in_maps = _prep_inputs(inputs)
    res = run_bass_kernel_spmd(nc, in_maps, core_ids=list(range(8)))
    y = np.empty((B, S, D), np.float32)
    for r in range(8):
        b, qq = r // 4, r % 4
        for tb in range(4):
            blk = 4 * tb + qq
            y[b, blk * 128:(blk + 1) * 128, :] = res.results[r]["y"][tb * 128:(tb + 1) * 128]
    return y


if __name__ == "__main__":
    data = np.load("/tmp/inputs.npz")
    inputs = {k: data[k] for k in data.files}
    out = kernel(**inputs)
    exp = np.load("/tmp/expected.npy")
    err = np.linalg.norm(out - exp) / np.linalg.norm(exp)
    print("Relative error:", err)


# revision 15
# speedup vs baseline: 1.7145x; 1.0601x over previous
"""BitAttention TRN2 kernel: 8-core SPMD (DP over batch x TP over kv-heads).

Self-contained: hardcodes shapes B=2, S=2048, D=2048, H=16, KH=4.
Core r: batch b = r//4, kv-head kh = r%4, stat/output stripe qq = r%4.

Math (forward-equivalent to the reference):
  - linear_bit = rms_norm -> per-row int8 act quant -> ternary weight quant
    -> matmul. Act-quant scale 127/max|xn| has rms self-cancel: the quantized
    ints are round(x*127/mx); rms enters only the per-token dequant scale.
  - Activations quantize straight to f16 with a +1536 offset (f16 ulp is 1 on
    [1024,2048) so the convert rounds half-to-even like jnp.round); the
    constant 1536 offset is removed inside the matmul by accumulating
    -1536*colsum(W) built from two exact f16 hi/lo matmuls.
  - Ternary weights quantize via round(clip(w*0.5/thr,-1,1)) (equivalent to
    round(tanh)), computed with the same +1536 f16 rounding trick.
  - The reference einsum sums the query-head group axis, so Q's 16 heads
    collapse to 4: group-sum the ternary w_q head blocks (ints in [-4,4]).
  - Scale folding: both 1/sqrt(HD) plus the wq/wk arctanh scales fold into
    the q rope tables (rope is linear); the wv scale cancels through the
    output rms-norm; the wo scale folds into the final dequant.
  - Attention runs transposed (S^T = K Q^T per key block) so softmaxed probs
    feed P^T directly into PV matmuls with no PE transposes; the softmax
    denominator comes from an extra all-ones column in the PV matmul.
    No max subtraction (logits are O(1) by construction).
  - Weight quantization work is sharded: batch-pair cores split w_q/w_k/w_v
    by row blocks, all 8 cores split w_o by columns; ternarized weights are
    exchanged with AllGather. Act-quant stats for x are sharded 4 ways across
    the batch group and AllGathered.
  - All tensor-engine transposes are done by DMA-transpose (f16/bf16).
"""
import numpy as np
from contextlib import ExitStack

import concourse.bass as bass
import concourse.bacc as bacc
import concourse.mybir as mybir
import concourse.tile as tile
from concourse.bass_utils import run_bass_kernel_spmd

B, S, D = 2, 2048, 2048
H, KH = 16, 4
HD = D // H          # 128
KVD = KH * HD        # 512
NB = S // 128        # 16 token blocks
SQ = S // 4          # 512 tokens per output stripe
EPS = 1e-8
MAGIC = float(1.5 * 2 ** 23)
M16 = 1536.0
ATANH05 = 0.5493061443340549      # arctanh(0.5)
NEG = -3.4e38
F32 = mybir.dt.float32
BF16 = mybir.dt.bfloat16
F16 = mybir.dt.float16
AX = mybir.AxisListType
OP = mybir.AluOpType
AF = mybir.ActivationFunctionType

_cache = {}


def _pt_off(kb, causal):
    if causal:
        return 2048 * kb - 64 * kb * (kb - 1)
    return 2048 * kb


def build(causal: bool, local_cc: bool = False):
    nc = bacc.Bacc()
    x_d = nc.dram_tensor("x", [S, D], F32, kind="ExternalInput")
    xs_d = nc.dram_tensor("xstat", [SQ, D], F32, kind="ExternalInput")
    wq_d = nc.dram_tensor("wq", [D // 2, KVD], F32, kind="ExternalInput")   # row-shard
    wk_d = nc.dram_tensor("wk", [D // 2, HD], F32, kind="ExternalInput")
    wv_d = nc.dram_tensor("wv", [D // 2, HD], F32, kind="ExternalInput")
    wo_d = nc.dram_tensor("wo", [KVD, D // 8], F32, kind="ExternalInput")   # col-shard
    cos_d = nc.dram_tensor("cos", [S, HD // 2], F32, kind="ExternalInput")
    sin_d = nc.dram_tensor("sin", [S, HD // 2], F32, kind="ExternalInput")
    qsel_d = nc.dram_tensor("qsel", [128, 2], F32, kind="ExternalInput")
    y_d = nc.dram_tensor("y", [SQ, D], F32, kind="ExternalOutput")
    st_in = nc.dram_tensor("st_in", [1, 4], F32)
    st_out = nc.dram_tensor("st_out", [1, 4], F32, addr_space="Shared")
    sc_in = nc.dram_tensor("sc_in", [4, 128, 2], F32)
    sc_out = nc.dram_tensor("sc_out", [4, 4, 128, 2], F32, addr_space="Shared")
    wg_in = nc.dram_tensor("wg_in", [8, 128, 3 * HD], F16)
    wg_out = nc.dram_tensor("wg_out", [2, 8, 128, 3 * HD], F16, addr_space="Shared")
    wob_in = nc.dram_tensor("wob_in", [4, 128, 256], BF16)
    wob_out = nc.dram_tensor("wob_out", [8, 4, 128, 256], BF16, addr_space="Shared")
    cc_in = nc.dram_tensor("cc_in", [4, 8, 128, HD], F32)
    cc_out = nc.dram_tensor("cc_out", [4, 8, 128, HD], F32, addr_space="Shared")

    PTW = _pt_off(NB, causal)

    with tile.TileContext(nc) as tc, ExitStack() as ctx:
        cpool = ctx.enter_context(tc.tile_pool(name="const", bufs=1))
        sm = ctx.enter_context(tc.tile_pool(name="sm", bufs=1))
        wres = ctx.enter_context(tc.tile_pool(name="wres", bufs=1))

        # ---------- constants ----------
        cmT = cpool.tile([128, 128], F32, tag="cmT")
        if causal:
            nc.gpsimd.memset(cmT[:], 0.0)
            nc.gpsimd.affine_select(out=cmT[:], in_=cmT[:], compare_op=OP.is_ge,
                                    fill=NEG, base=0, pattern=[[1, 128]],
                                    channel_multiplier=-1)
        ones_f16 = cpool.tile([128, 1], F16, tag="o16")
        nc.any.memset(ones_f16[:], 1.0)
        ones_bf = cpool.tile([128, 1], BF16, tag="obf")
        nc.any.memset(ones_bf[:], 1.0)
        ones_c = cpool.tile([128, 1], F32, tag="onc")
        nc.any.memset(ones_c[:], 1.0)
        ones_r = cpool.tile([1, 128], F32, tag="onr")
        nc.any.memset(ones_r[:], 1.0)
        n192 = cpool.tile([128, 128], F16, tag="n192")
        nc.any.memset(n192[:], -192.0)
        n12 = cpool.tile([128, 128], F16, tag="n12")
        nc.any.memset(n12[:], -12.0)
        inv_n = cpool.tile([128, 4], F32, tag="invn")
        for j, numel in enumerate([D * D, KVD * D, KVD * D, D * KVD]):
            nc.any.memset(inv_n[:, j:j + 1], 1.0 / numel)
        sqscr = cpool.tile([128, D], BF16, tag="sqscr")
        cos_kb = cpool.tile([128, NB, HD // 2], BF16, tag="coskb")
        sin_kb = cpool.tile([128, NB, HD // 2], BF16, tag="sinkb")
        cos_qb = cpool.tile([128, NB, HD // 2], BF16, tag="cosqb")
        sin_qb = cpool.tile([128, NB, HD // 2], BF16, tag="sinqb")

        # persistent smalls
        pr = sm.tile([128, 4], F32, tag="pr")
        st_sb = sm.tile([1, 4], F32, tag="st_sb")
        st2_sb = sm.tile([1, 4], F32, tag="st2_sb")
        totals = sm.tile([128, 4], F32, tag="totals")
        s4 = sm.tile([128, 4], F32, tag="s4")
        hi4 = sm.tile([128, 4], F32, tag="hi4")
        a4 = sm.tile([128, 4], F32, tag="a4")
        aqk = sm.tile([128, 1], F32, tag="aqk")
        mxs = sm.tile([128, 4], F32, tag="mxs")
        ssqs = sm.tile([128, 4], F32, tag="ssqs")
        sx_sb = sm.tile([128, 4, 2], F32, tag="sx_sb")
        sd_all = sm.tile([128, 16, 2], F32, tag="sd_all")
        csr = sm.tile([1, 3 * HD], F32, tag="csr")
        csbc = sm.tile([128, 3 * HD], F32, tag="csbc")
        hq = sm.tile([128, 3 * HD], F16, tag="hq")
        lq = sm.tile([128, 3 * HD], F16, tag="lq")

        # persistent quantized weights
        wqkv = wres.tile([128, NB, 3 * HD], F16, tag="wqkv", name="wqkv")
        wo_q = wres.tile([128, 4, D], BF16, tag="wo_q", name="wo_q")

        qsel = cpool.tile([128, 2], F32, tag="qsel")

        # long-lived x streaming pool + attention-input tiles
        xph = ctx.enter_context(tc.tile_pool(name="xph", bufs=1))
        atl = ctx.enter_context(tc.tile_pool(name="atl", bufs=1))
        qT = atl.tile([128, NB, 128], BF16, tag="qT", name="qT")
        kT = atl.tile([128, NB, 128], BF16, tag="kT", name="kT")
        qkv_all = sm.tile([128, NB, 3 * HD], BF16, tag="qkv_all", name="qkv_all")

        def half_quant(qch, ib, i, xb):
            hw = D // 2
            nc.vector.tensor_scalar(qch[:, ib, 0:hw], xb[:, 0:hw],
                                    sd_all[:, i, 0:1], M16, op0=OP.mult, op1=OP.add)
            nc.scalar.activation(qch[:, ib, hw:D], xb[:, hw:D], AF.Copy,
                                 bias=M16, scale=sd_all[:, i, 0:1])

        def rope_chunk(ci, qr, kr):
            hh = HD // 2
            cs = slice(4 * ci, 4 * ci + 4)
            for src0, cosb, sinb, dst in ((0, cos_qb, sin_qb, qr),
                                          (HD, cos_kb, sin_kb, kr)):
                ev = qkv_all[:, cs, src0:src0 + hh]
                od = qkv_all[:, cs, src0 + hh:src0 + HD]
                t1 = xph.tile([128, 4, hh], BF16, tag="t1", bufs=2)
                t2 = xph.tile([128, 4, hh], BF16, tag="t2", bufs=2)
                nc.vector.tensor_tensor(t1[:], ev, cosb[:, cs, :], op=OP.mult)
                nc.vector.tensor_tensor(t2[:], od, sinb[:, cs, :], op=OP.mult)
                nc.vector.tensor_tensor(dst[:, cs, 0:hh], t1[:], t2[:], op=OP.subtract)
                t3 = xph.tile([128, 4, hh], BF16, tag="t1", bufs=2)
                t4 = xph.tile([128, 4, hh], BF16, tag="t2", bufs=2)
                nc.vector.tensor_tensor(t3[:], ev, sinb[:, cs, :], op=OP.mult)
                nc.vector.tensor_tensor(t4[:], od, cosb[:, cs, :], op=OP.mult)
                nc.vector.tensor_tensor(dst[:, cs, hh:HD], t3[:], t4[:], op=OP.add)
            nc.sync.dma_start_transpose(
                qT[:, cs, :], qr[:, cs, :].rearrange("p a b -> p (a b)"))
            nc.sync.dma_start_transpose(
                kT[:, cs, :], kr[:, cs, :].rearrange("p a b -> p (a b)"))

        with tc.tile_pool(name="cstage", bufs=1) as cstage:
          with tc.tile_pool(name="wph", bufs=1) as wph:
            # ---- all input DMAs up front (SP in readiness order) ----
            cosf = cstage.tile([128, NB, HD // 2], F32, tag="cosf")
            sinf = cstage.tile([128, NB, HD // 2], F32, tag="sinf")
            nc.sync.dma_start(cosf[:], cos_d.ap().rearrange("(i p) f -> p i f", p=128))
            nc.sync.dma_start(sinf[:], sin_d.ap().rearrange("(i p) f -> p i f", p=128))
            wq_sb = wph.tile([128, 8, KVD], F32, tag="wq_sb")
            wk_sb = wph.tile([128, 8, HD], F32, tag="wk_sb")
            wv_sb = wph.tile([128, 8, HD], F32, tag="wv_sb")
            wo_sb = wph.tile([128, 4, 256], F32, tag="wo_sb")
            nc.sync.dma_start(wq_sb[:], wq_d.ap().rearrange("(j p) c -> p j c", p=128))
            nc.sync.dma_start(wk_sb[:], wk_d.ap().rearrange("(j p) c -> p j c", p=128))
            nc.sync.dma_start(wv_sb[:], wv_d.ap().rearrange("(j p) c -> p j c", p=128))
            nc.sync.dma_start(wo_sb[:], wo_d.ap().rearrange("(c p) d -> p c d", p=128))
            xstat = [xph.tile([128, D], F32, tag="xb", bufs=3, name=f"xst{i}")
                     for i in range(4)]
            for i in range(4):
                nc.sync.dma_start(xstat[i][:], xs_d[i * 128:(i + 1) * 128, :])
            nc.sync.dma_start(qsel[:], qsel_d[:])
            nc.gpsimd.tensor_copy(cos_kb[:], cosf[:])
            nc.gpsimd.tensor_copy(sin_kb[:], sinf[:])

            # ---- pass1 |w| row sums, then x stats ----
            nc.vector.tensor_reduce(pr[:, 0:1], wq_sb[:].rearrange("p a b -> p (a b)"),
                                    axis=AX.X, op=OP.add, apply_absolute_value=True)
            nc.vector.tensor_reduce(pr[:, 1:2], wk_sb[:].rearrange("p a b -> p (a b)"),
                                    axis=AX.X, op=OP.add, apply_absolute_value=True)
            nc.vector.tensor_reduce(pr[:, 2:3], wv_sb[:].rearrange("p a b -> p (a b)"),
                                    axis=AX.X, op=OP.add, apply_absolute_value=True)
            nc.vector.tensor_reduce(pr[:, 3:4], wo_sb[:].rearrange("p a b -> p (a b)"),
                                    axis=AX.X, op=OP.add, apply_absolute_value=True)
            for i in range(4):
                nc.vector.tensor_reduce(mxs[:, i:i + 1], xstat[i][:], axis=AX.X,
                                        op=OP.max, apply_absolute_value=True)
                nc.scalar.activation(sqscr[:], xstat[i][:], AF.Square,
                                     accum_out=ssqs[:, i:i + 1])

            # ---- x act-quant scalar chain + exchange (SP) ----
            mean4 = sm.tile([128, 4], F32, tag="mean4")
            nc.vector.tensor_scalar(mean4[:], ssqs[:], 1.0 / D, EPS, op0=OP.mult, op1=OP.add)
            sd4 = sm.tile([128, 4], F32, tag="sd4")
            nc.scalar.activation(sd4[:], mean4[:], AF.Sqrt)
            r4 = sm.tile([128, 4], F32, tag="r4")
            nc.vector.reciprocal(r4[:], sd4[:])
            nt4 = sm.tile([128, 4], F32, tag="nt4")
            nc.vector.tensor_tensor(nt4[:], sd4[:], r4[:], op=OP.mult)
            nc.vector.tensor_scalar(nt4[:], nt4[:], -1.0, 2.0, op0=OP.mult, op1=OP.add)
            nc.vector.tensor_tensor(r4[:], r4[:], nt4[:], op=OP.mult)
            m127 = sm.tile([128, 4], F32, tag="m127")
            nc.vector.tensor_scalar(m127[:], mxs[:], 1.0 / 127.0, None, op0=OP.mult)
            smul4 = sm.tile([128, 4], F32, tag="smul4")
            nc.vector.reciprocal(smul4[:], m127[:])
            nc.vector.tensor_tensor(nt4[:], m127[:], smul4[:], op=OP.mult)
            nc.vector.tensor_scalar(nt4[:], nt4[:], -1.0, 2.0, op0=OP.mult, op1=OP.add)
            nc.vector.tensor_tensor(smul4[:], smul4[:], nt4[:], op=OP.mult)
            deq4 = sm.tile([128, 4], F32, tag="deq4")
            nc.vector.tensor_tensor(deq4[:], mxs[:], r4[:], op=OP.mult)
            nc.vector.tensor_scalar(deq4[:], deq4[:], 1.0 / 127.0, None, op0=OP.mult)
            nc.vector.tensor_copy(sx_sb[:, :, 0], smul4[:])
            nc.vector.tensor_copy(sx_sb[:, :, 1], deq4[:])
            nc.sync.dma_start(sc_in.ap().rearrange("i p c -> p i c"), sx_sb[:])
            if local_cc:
                nc.sync.dma_start(sc_out.ap()[0], sc_in.ap())
            else:
                nc.gpsimd.collective_compute(
                    "AllGather", OP.bypass,
                    replica_groups=[[0, 1, 2, 3], [4, 5, 6, 7]],
                    ins=[sc_in.ap().opt()], outs=[sc_out.ap().opt()])
            nc.sync.dma_start(sd_all[:], sc_out.ap().rearrange("s i p c -> p (s i) c"))

            # ---- weight stats reduce + exchange (SP) ----
            with tc.tile_pool(name="psst", bufs=2, space="PSUM") as psst:
                pcol = psst.tile([1, 4], F32, tag="st")
                nc.tensor.matmul(pcol[:], ones_c[:], pr[:], start=True, stop=True)
                nc.vector.tensor_copy(st_sb[:], pcol[:])
                nc.sync.dma_start(st_in[:], st_sb[:])
                if local_cc:
                    nc.sync.dma_start(st_out.ap(), st_in.ap())
                else:
                    nc.gpsimd.collective_compute(
                        "AllReduce", OP.add, replica_groups=[list(range(8))],
                        ins=[st_in.ap().opt()], outs=[st_out.ap().opt()])
                nc.sync.dma_start(st2_sb[:], st_out[:])
                bc = psst.tile([128, 4], F32, tag="st")
                nc.tensor.matmul(bc[:], ones_r[:], st2_sb[:], start=True, stop=True)
                nc.vector.tensor_copy(totals[:], bc[:])

            # ---- ternary thresholds and scales ----
            nc.vector.tensor_tensor(s4[:], totals[:], inv_n[:], op=OP.mult)
            thr2 = sm.tile([128, 4], F32, tag="thr2")
            nc.vector.tensor_scalar(thr2[:], s4[:], EPS, 2.0 * ATANH05,
                                    op0=OP.add, op1=OP.mult)
            nc.vector.reciprocal(hi4[:], thr2[:])
            ntp = sm.tile([128, 4], F32, tag="ntp")
            nc.vector.tensor_tensor(ntp[:], thr2[:], hi4[:], op=OP.mult)
            nc.vector.tensor_scalar(ntp[:], ntp[:], -1.0, 2.0, op0=OP.mult, op1=OP.add)
            nc.vector.tensor_tensor(hi4[:], hi4[:], ntp[:], op=OP.mult)
            num = sm.tile([128, 4], F32, tag="num")
            den = sm.tile([128, 4], F32, tag="den")
            rat = sm.tile([128, 4], F32, tag="rat")
            nc.vector.tensor_scalar(num[:], s4[:], 1.0, None, op0=OP.add)
            nc.vector.tensor_scalar(den[:], s4[:], -1.0, 1.0, op0=OP.mult, op1=OP.add)
            nc.vector.reciprocal(rat[:], den[:])
            nc.vector.tensor_tensor(ntp[:], den[:], rat[:], op=OP.mult)
            nc.vector.tensor_scalar(ntp[:], ntp[:], -1.0, 2.0, op0=OP.mult, op1=OP.add)
            nc.vector.tensor_tensor(rat[:], rat[:], ntp[:], op=OP.mult)
            nc.vector.tensor_tensor(rat[:], rat[:], num[:], op=OP.mult)
            lnr = sm.tile([128, 4], F32, tag="lnr")
            nc.scalar.activation(lnr[:], rat[:], AF.Ln)
            nc.vector.tensor_scalar(a4[:], lnr[:], 0.5, None, op0=OP.mult)
            nc.vector.tensor_tensor(aqk[:], a4[:, 0:1], a4[:, 1:2], op=OP.mult)
            nc.vector.tensor_scalar(aqk[:], aqk[:], 1.0 / HD, None, op0=OP.mult)
            nc.vector.tensor_scalar(cos_qb[:], cosf[:], aqk[:], None, op0=OP.mult)
            nc.vector.tensor_scalar(sin_qb[:], sinf[:], aqk[:], None, op0=OP.mult)

            # ---- ternary quantize shards ----
            wsh = wph.tile([128, 8, 3 * HD], F16, tag="wsh")
            wosh = wph.tile([128, 4, 256], BF16, tag="wosh")
            tq = wph.tile([128, 8, KVD], F16, tag="tq")
            tk = wph.tile([128, 8, HD], F16, tag="tk")
            tg1 = wph.tile([128, 8, HD], F16, tag="tg1")
            tg2 = wph.tile([128, 8, HD], F16, tag="tg2")
            nc.vector.tensor_scalar(wq_sb[:], wq_sb[:], hi4[:, 0:1], 1.0,
                                    op0=OP.mult, op1=OP.min)
            nc.vector.tensor_scalar(tq[:], wq_sb[:], -1.0, M16, op0=OP.max, op1=OP.add)
            nc.vector.tensor_scalar(tq[:], tq[:], M16, None, op0=OP.subtract)
            nc.vector.tensor_tensor(tg1[:], tq[:, :, 0:HD], tq[:, :, HD:2 * HD], op=OP.add)
            nc.vector.tensor_tensor(tg2[:], tq[:, :, 2 * HD:3 * HD], tq[:, :, 3 * HD:4 * HD], op=OP.add)
            nc.vector.tensor_tensor(wsh[:, :, 0:HD], tg1[:], tg2[:], op=OP.add)
            nc.gpsimd.tensor_scalar(wk_sb[:], wk_sb[:], hi4[:, 1:2], 1.0,
                                    op0=OP.mult, op1=OP.min)
            nc.vector.tensor_scalar(tk[:], wk_sb[:], -1.0, M16, op0=OP.max, op1=OP.add)
            nc.vector.tensor_scalar(wsh[:, :, HD:2 * HD], tk[:], M16, None, op0=OP.subtract)
            nc.gpsimd.tensor_scalar(wv_sb[:], wv_sb[:], hi4[:, 2:3], 1.0,
                                    op0=OP.mult, op1=OP.min)
            nc.vector.tensor_scalar(tk[:], wv_sb[:], -1.0, M16, op0=OP.max, op1=OP.add)
            nc.vector.tensor_scalar(wsh[:, :, 2 * HD:3 * HD], tk[:], M16, None, op0=OP.subtract)
            nc.gpsimd.tensor_scalar(wo_sb[:], wo_sb[:], hi4[:, 3:4], 1.0,
                                    op0=OP.mult, op1=OP.min)
            to16 = wph.tile([128, 4, 256], F16, tag="to16")
            nc.vector.tensor_scalar(to16[:], wo_sb[:], -1.0, M16, op0=OP.max, op1=OP.add)
            nc.vector.tensor_scalar(wosh[:], to16[:], M16, None, op0=OP.subtract)

            # ---- first x chunk quantizes while the exchange runs ----
            qch0 = xph.tile([128, 4, D], F16, tag="qch", bufs=2)
            xbs0 = []
            for ib in range(4):
                xb = xph.tile([128, D], F32, tag="xb", bufs=3)
                nc.sync.dma_start(xb[:], x_d[ib * 128:(ib + 1) * 128, :])
                xbs0.append(xb)
            for ib in range(4):
                half_quant(qch0, ib, ib, xbs0[ib])

            # ---- exchange ternary shards (ACT-issued) ----
            nc.scalar.dma_start(wg_in.ap().rearrange("j p c -> p j c"), wsh[:])
            nc.scalar.dma_start(wob_in.ap().rearrange("c p d -> p c d"), wosh[:])
            if local_cc:
                nc.scalar.dma_start(wg_out.ap()[0], wg_in.ap())
            else:
                nc.gpsimd.collective_compute(
                    "AllGather", OP.bypass,
                    replica_groups=[[0, 4], [1, 5], [2, 6], [3, 7]],
                    ins=[wg_in.ap().opt()], outs=[wg_out.ap().opt()])
            for s in range(2):
                nc.scalar.dma_start(
                    wqkv[:].rearrange("p (j s) c -> p j s c", s=2)[:, :, s, :],
                    wg_out.ap()[s].rearrange("j p c -> p j c"))
          # (wph closed: f32 weight shards freed)

        # ---------- x quantize/transpose fused with QKV ----------
        with tc.tile_pool(name="xqTp", bufs=1) as xqTp, \
             tc.tile_pool(name="psc", bufs=3, space="PSUM") as psc:
            xqT = xqTp.tile([128, NB * NB, 128], F16, tag="xqT")
            qr = xqTp.tile([128, NB, HD], BF16, tag="qr")
            kr = xqTp.tile([128, NB, HD], BF16, tag="kr")
            nc.sync.dma_start_transpose(xqT[:, 0:64, :],
                                        qch0[:].rearrange("p a b -> p (a b)"))

            # csum of wqkv for the -1536 offset correction (f16 hi/lo split)
            csp = psc.tile([1, 3 * HD], F32, tag="cs", bufs=2)
            for j in range(NB):
                nc.tensor.matmul(csp[:], ones_f16[:], wqkv[:, j, :],
                                 start=(j == 0), stop=(j == NB - 1))
            nc.scalar.activation(csr[:], csp[:], AF.Copy)
            bcp = psc.tile([128, 3 * HD], F32, tag="cs", bufs=2)
            nc.tensor.matmul(bcp[:], ones_r[:], csr[:], start=True, stop=True)
            nc.scalar.activation(csbc[:], bcp[:], AF.Copy)
            tcs = sm.tile([128, 3 * HD], F32, tag="tcs")
            nc.gpsimd.tensor_scalar(tcs[:], csbc[:], 1.0 / 16.0, MAGIC,
                                    op0=OP.mult, op1=OP.add)
            nc.gpsimd.tensor_scalar(tcs[:], tcs[:], MAGIC, None, op0=OP.subtract)
            nc.gpsimd.tensor_copy(hq[:], tcs[:])
            nc.gpsimd.scalar_tensor_tensor(lq[:], tcs[:], -16.0, csbc[:],
                                           op0=OP.mult, op1=OP.add)

            def qkv_chunk(ci):
                for ib in range(4):
                    i = 4 * ci + ib
                    pq = psc.tile([128, 3 * HD], F32, tag="mm", bufs=3)
                    for j in range(NB):
                        nc.tensor.matmul(pq[:], xqT[:, 16 * i + j, :],
                                         wqkv[:, j, :], start=(j == 0), stop=False)
                    nc.tensor.matmul(pq[:], n192[:], hq[:], start=False, stop=False,
                                     skip_group_check=True)
                    nc.tensor.matmul(pq[:], n12[:], lq[:], start=False, stop=True,
                                     skip_group_check=True)
                    nc.vector.tensor_scalar(qkv_all[:, i, :], pq[:],
                                            sd_all[:, i, 1:2], None, op0=OP.mult)

            qkv_chunk(0)
            for ci in range(1, 4):
                qch = xph.tile([128, 4, D], F16, tag="qch", bufs=2)
                for ib in range(4):
                    i = 4 * ci + ib
                    xb = xph.tile([128, D], F32, tag="xb", bufs=3)
                    nc.sync.dma_start(xb[:], x_d[i * 128:(i + 1) * 128, :])
                    half_quant(qch, ib, i, xb)
                nc.sync.dma_start_transpose(xqT[:, 64 * ci:64 * (ci + 1), :],
                                            qch[:].rearrange("p a b -> p (a b)"))
                qkv_chunk(ci)
                rope_chunk(ci - 1, qr, kr)
            rope_chunk(3, qr, kr)

        # finish the wo gather (needed only by the output projection)
        if local_cc:
            nc.scalar.dma_start(wob_out.ap()[0], wob_in.ap())
        else:
            nc.gpsimd.collective_compute(
                "AllGather", OP.bypass, replica_groups=[list(range(8))],
                ins=[wob_in.ap().opt()], outs=[wob_out.ap().opt()])
        for s in range(8):
            nc.scalar.dma_start(
                wo_q[:].rearrange("p c (s d) -> p c s d", s=8)[:, :, s, :],
                wob_out.ap()[s].rearrange("c p d -> p c d"))

        # ---------- attention + output projection ----------
        qTf = qT[:].rearrange("p a b -> p (a b)")
        with tc.tile_pool(name="attn", bufs=1) as attn, \
             tc.tile_pool(name="pss", bufs=3, space="PSUM") as pss, \
             tc.tile_pool(name="psv", bufs=2, space="PSUM") as psv, \
             tc.tile_pool(name="psy", bufs=2, space="PSUM") as psy:
            PT = attn.tile([128, PTW], BF16, tag="PT")
            obuf = attn.tile([128, 4, HD], F32, tag="obuf", bufs=2)

            def scores(kb):
                qlo = 128 * kb if causal else 0
                c0 = qlo
                first = True
                while c0 < S:
                    cw = min(512, S - c0)
                    sp = pss.tile([128, 512], F32, tag="sc")
                    nc.tensor.matmul(sp[:, 0:cw], kT[:, kb, :], qTf[:, c0:c0 + cw],
                                     start=True, stop=True)
                    if causal and first:
                        nc.vector.tensor_tensor(sp[:, 0:128], sp[:, 0:128], cmT[:],
                                                op=OP.add)
                    nc.scalar.activation(PT[:, _pt_off(kb, causal) + c0 - qlo:
                                            _pt_off(kb, causal) + c0 - qlo + cw],
                                         sp[:, 0:cw], AF.Exp)
                    first = False
                    c0 += cw

            def pv(qb):
                po = psv.tile([128, 132], F32, tag="po")
                nkb = qb + 1 if causal else NB
                for k2 in range(nkb):
                    qoff = (qb - k2) * 128 if causal else qb * 128
                    lhs = PT[:, _pt_off(k2, causal) + qoff:
                             _pt_off(k2, causal) + qoff + 128]
                    nc.tensor.matmul(po[:, 0:HD], lhs, qkv_all[:, k2, 2 * HD:3 * HD],
                                     start=(k2 == 0), stop=(k2 == nkb - 1),
                                     skip_group_check=True)
                    nc.tensor.matmul(po[:, HD:HD + 1], lhs, ones_bf[:],
                                     start=(k2 == 0), stop=(k2 == nkb - 1),
                                     skip_group_check=True)
                rz = attn.tile([128, 1], F32, tag="rz", bufs=2)
                nz = attn.tile([128, 1], F32, tag="nz", bufs=2)
                nc.vector.reciprocal(rz[:], po[:, HD:HD + 1])
                nc.vector.tensor_tensor(nz[:], po[:, HD:HD + 1], rz[:], op=OP.mult)
                nc.vector.tensor_scalar(nz[:], nz[:], -1.0, 2.0, op0=OP.mult, op1=OP.add)
                nc.vector.tensor_tensor(rz[:], rz[:], nz[:], op=OP.mult)
                nc.scalar.activation(obuf[:, qb % 4, :], po[:, 0:HD], AF.Copy,
                                     scale=rz[:])

            xo8s = [None] * 4

            def cc_ex(tb):
                nc.sync.dma_start(cc_in.ap()[tb, 0:4].rearrange("s p d -> p s d"),
                                  obuf[:])
                nc.sync.dma_start(cc_in.ap()[tb, 4:8].rearrange("s p d -> p s d"),
                                  obuf[:])
                if local_cc:
                    nc.sync.dma_start(cc_out.ap()[tb], cc_in.ap()[tb])
                else:
                    nc.gpsimd.collective_compute(
                        "AllToAll", OP.bypass, replica_groups=[list(range(8))],
                        ins=[cc_in.ap()[tb].opt()], outs=[cc_out.ap()[tb].opt()])
                xo8 = attn.tile([128, 8, HD], F32, tag="xo8", bufs=2)
                nc.sync.dma_start(xo8[:], cc_out.ap()[tb].rearrange("s p d -> p s d"))
                xo8s[tb] = xo8

            def oproj(tb):
                xo8 = xo8s[tb]
                xsel = attn.tile([128, KVD], F32, tag="xsel", bufs=2)
                nc.gpsimd.tensor_scalar(xsel[:], xo8[:, 0:4, :].rearrange("p a b -> p (a b)"),
                                        qsel[:, 0:1], None, op0=OP.mult)
                xo = attn.tile([128, KVD], F32, tag="xo", bufs=2)
                nc.gpsimd.scalar_tensor_tensor(xo[:], xo8[:, 4:8, :].rearrange("p a b -> p (a b)"),
                                               qsel[:, 1:2], xsel[:],
                                               op0=OP.mult, op1=OP.add)
                mx2 = attn.tile([128, 1], F32, tag="mx2", bufs=2)
                nc.vector.tensor_reduce(mx2[:], xo[:], axis=AX.X, op=OP.max,
                                        apply_absolute_value=True)
                ssq2 = attn.tile([128, 1], F32, tag="ssq2", bufs=2)
                nc.scalar.activation(sqscr[:, 0:KVD], xo[:], AF.Square, accum_out=ssq2[:])
                mean2 = attn.tile([128, 1], F32, tag="mean2", bufs=2)
                nc.vector.tensor_scalar(mean2[:], ssq2[:], 1.0 / KVD, EPS,
                                        op0=OP.mult, op1=OP.add)
                sd2 = attn.tile([128, 1], F32, tag="sd2", bufs=2)
                nc.scalar.activation(sd2[:], mean2[:], AF.Sqrt)
                r2 = attn.tile([128, 1], F32, tag="r2", bufs=2)
                nt2 = attn.tile([128, 1], F32, tag="nt2", bufs=2)
                nc.vector.reciprocal(r2[:], sd2[:])
                nc.vector.tensor_tensor(nt2[:], sd2[:], r2[:], op=OP.mult)
                nc.vector.tensor_scalar(nt2[:], nt2[:], -1.0, 2.0, op0=OP.mult, op1=OP.add)
                nc.vector.tensor_tensor(r2[:], r2[:], nt2[:], op=OP.mult)
                m2 = attn.tile([128, 1], F32, tag="m2", bufs=2)
                nc.vector.tensor_scalar(m2[:], mx2[:], 1.0 / 127.0, None, op0=OP.mult)
                sl2 = attn.tile([128, 1], F32, tag="sl2", bufs=2)
                nc.vector.reciprocal(sl2[:], m2[:])
                nc.vector.tensor_tensor(nt2[:], m2[:], sl2[:], op=OP.mult)
                nc.vector.tensor_scalar(nt2[:], nt2[:], -1.0, 2.0, op0=OP.mult, op1=OP.add)
                nc.vector.tensor_tensor(sl2[:], sl2[:], nt2[:], op=OP.mult)
                dqy = attn.tile([128, 1], F32, tag="dqy", bufs=2)
                nc.vector.tensor_tensor(dqy[:], mx2[:], r2[:], op=OP.mult)
                nc.vector.tensor_scalar(dqy[:], dqy[:], 1.0 / 127.0, None, op0=OP.mult)
                nc.vector.tensor_tensor(dqy[:], dqy[:], a4[:, 3:4], op=OP.mult)
                nc.vector.tensor_scalar(xo[:], xo[:], sl2[:], MAGIC,
                                        op0=OP.mult, op1=OP.add)
                qo = attn.tile([128, KVD], BF16, tag="qo", bufs=2)
                nc.scalar.activation(qo[:], xo[:], AF.Copy, bias=-MAGIC)
                xoT = attn.tile([128, 4, 128], BF16, tag="xoT", bufs=2)
                nc.sync.dma_start_transpose(xoT[:], qo[:])
                y_sb = attn.tile([128, D], F32, tag="ysb", bufs=2)
                for oc in range(4):
                    py = psy.tile([128, 512], F32, tag="my")
                    for jc in range(4):
                        nc.tensor.matmul(py[:], xoT[:, jc, :],
                                         wo_q[:, jc, oc * 512:(oc + 1) * 512],
                                         start=(jc == 0), stop=(jc == 3))
                    if oc % 2 == 0:
                        nc.scalar.activation(y_sb[:, oc * 512:(oc + 1) * 512], py[:],
                                             AF.Copy, scale=dqy[:])
                    else:
                        nc.vector.tensor_scalar(y_sb[:, oc * 512:(oc + 1) * 512],
                                                py[:], dqy[:], None, op0=OP.mult)
                nc.sync.dma_start(y_d[tb * 128:(tb + 1) * 128, :], y_sb[:])

            def post_pv(qb):
                # at quarter completion: launch the exchange; run the PREVIOUS
                # quarter's output projection (its data has long arrived)
                if qb % 4 == 3:
                    tb = qb // 4
                    cc_ex(tb)
                    if tb >= 1:
                        oproj(tb - 1)

            if causal:
                scores(0)
                for kb in range(1, NB):
                    scores(kb)
                    pv(kb - 1)
                    post_pv(kb - 1)
                pv(NB - 1)
                post_pv(NB - 1)
            else:
                for kb in range(NB):
                    scores(kb)
                for qb in range(NB):
                    pv(qb)
                    post_pv(qb)
            oproj(3)
    nc.compile()
    return nc


def _rope_perm():
    p = np.empty(HD, np.int64)
    p[:HD // 2] = np.arange(0, HD, 2)
    p[HD // 2:] = np.arange(1, HD, 2)
    return p


def qsel_host(b):
    q = np.zeros((128, 2), np.float32)
    q[:, b] = 1.0
    return q


def _prep_inputs(inputs):
    x = np.ascontiguousarray(np.asarray(inputs["x"], np.float32))
    w_q = np.asarray(inputs["w_q"], np.float32)
    w_k = np.asarray(inputs["w_k"], np.float32)
    w_v = np.asarray(inputs["w_v"], np.float32)
    w_o = np.asarray(inputs["w_o"], np.float32)
    cos = np.ascontiguousarray(np.asarray(inputs["freq_cos"], np.float32))
    sin = np.ascontiguousarray(np.asarray(inputs["freq_sin"], np.float32))
    perm = _rope_perm()
    woT = np.ascontiguousarray(w_o.T)                      # [KVD, D]
    in_maps = []
    jrows = np.arange(D) // 128 % 2
    for r in range(8):
        b, kh = r // 4, r % 4
        heads = [g * KH + kh for g in range(4)]
        wq_sel = w_q.reshape(H, HD, D)[heads][:, perm, :]  # [4,128,D]
        wqT = np.ascontiguousarray(wq_sel.reshape(4 * HD, D).T)   # [D, 512]
        wkT = np.ascontiguousarray(w_k[kh * HD:(kh + 1) * HD][perm].T)  # [D,128]
        wvT = np.ascontiguousarray(w_v[kh * HD:(kh + 1) * HD].T)        # [D,128]
        sel = jrows == b
        in_maps.append({
            "x": x[b],
            "xstat": np.ascontiguousarray(x[b][kh * SQ:(kh + 1) * SQ]),
            "wq": np.ascontiguousarray(wqT[sel]),
            "wk": np.ascontiguousarray(wkT[sel]),
            "wv": np.ascontiguousarray(wvT[sel]),
            "wo": np.ascontiguousarray(woT[:, r * 256:(r + 1) * 256]),
            "cos": cos, "sin": sin,
            "qsel": qsel_host(b),
        })
    return in_maps


def _gains_trivial(inputs):
    return all(np.all(np.asarray(inputs[g]) == 1.0)
               for g in ("g_q", "g_k", "g_v", "g_o"))


def _numpy_fallback(inputs):
    """Faithful numpy reimplementation (slow); used only for unexpected configs."""
    x = np.asarray(inputs["x"], np.float32)
    cos, sin = (np.asarray(inputs[k], np.float32) for k in ("freq_cos", "freq_sin"))
    causal = int(np.asarray(inputs["causal"]))

    def rms(t, g):
        n = t * (1.0 / np.sqrt(np.mean(t * t, -1, keepdims=True, dtype=np.float32) + EPS))
        return (g * n).astype(np.float32)

    def actq(t):
        scale = 127.0 / np.clip(np.max(np.abs(t), -1, keepdims=True), 1e-4, None)
        q = np.round(t * scale)
        return np.clip(q, -128, 127) / scale

    def ternq(w):
        s = np.mean(np.abs(w), dtype=np.float32)
        return np.round(np.tanh(w / (s + EPS))) * np.arctanh(s)

    def lin(t, w, g):
        return actq(rms(t, g)).astype(np.float32) @ ternq(np.asarray(w, np.float32)).T

    Bb, Ss, Dd = x.shape
    q = lin(x, inputs["w_q"], np.asarray(inputs["g_q"], np.float32)).reshape(Bb, Ss, H, HD)
    k = lin(x, inputs["w_k"], np.asarray(inputs["g_k"], np.float32)).reshape(Bb, Ss, KH, HD)
    v = lin(x, inputs["w_v"], np.asarray(inputs["g_v"], np.float32)).reshape(Bb, Ss, KH, HD)

    def rope(t):
        t2 = t.reshape(*t.shape[:-1], -1, 2)
        c = cos[None, :, None, :]
        s_ = sin[None, :, None, :]
        o0 = t2[..., 0] * c - t2[..., 1] * s_
        o1 = t2[..., 0] * s_ + t2[..., 1] * c
        return np.stack([o0, o1], -1).reshape(t.shape).astype(np.float32)

    q, k = rope(q), rope(k)
    scale = np.float32(HD ** 0.5)
    q = q.transpose(0, 2, 1, 3) / scale
    k = k.transpose(0, 2, 1, 3)
    v = v.transpose(0, 2, 1, 3)
    qg = q.reshape(Bb, 4, KH, Ss, HD).sum(1)
    sc = np.einsum("bhnd,bhsd->bhns", qg, k).astype(np.float32)
    if causal:
        mask = np.tril(np.ones((Ss, Ss), bool))
        sc = np.where(mask[None, None], sc, np.float32(np.finfo(np.float32).min))
    sc = sc / scale
    sc = sc - sc.max(-1, keepdims=True)
    p = np.exp(sc)
    p /= p.sum(-1, keepdims=True)
    out = np.einsum("bhns,bhsd->bnhd", p, v).reshape(Bb, Ss, KVD)
    return lin(out, inputs["w_o"], np.asarray(inputs["g_o"], np.float32))


def kernel(**inputs):
    x = np.asarray(inputs["x"])
    if x.shape != (B, S, D) or not _gains_trivial(inputs):
        return _numpy_fallback(inputs)
    causal = bool(int(np.asarray(inputs["causal"])))
    key = ("bitattn", causal)
    if key not in _cache:
        _cache[key] = build(causal)
    nc = _cache[key]
    in_maps = _prep_inputs(inputs)
    res = run_bass_kernel_spmd(nc, in_maps, core_ids=list(range(8)))
    y = np.empty((B, S, D), np.float32)
    for r in range(8):
        b, qq = r // 4, r % 4
        for tb in range(4):
            blk = 4 * tb + qq
            y[b, blk * 128:(blk + 1) * 128, :] = res.results[r]["y"][tb * 128:(tb + 1) * 128]
    return y


if __name__ == "__main__":
    data = np.load("/tmp/inputs.npz")
    inputs = {k: data[k] for k in data.files}
    out = kernel(**inputs)
    exp = np.load("/tmp/expected.npy")
    err = np.linalg.norm(out - exp) / np.linalg.norm(exp)
    print("Relative error:", err)
